# revision 2
# baseline (speedup 1.0000x reference)
"""GQA (B=1, S=2048, D=2048, 32 Q heads / 8 KV heads, head_dim=64, RoPE,
non-causal softmax) on 8 Trainium2 NeuronCores.

Sharding: tensor-parallel over heads. Core c owns Q heads 4c..4c+3 and KV head c.
Each core computes y_c = softmax(q_c k_c^T / 8) v_c @ Wo[:, c*256:(c+1)*256].T
(a full [S, D] partial, bf16); the host sums the 8 partials.

v2 (vs f32r baseline):
  - bf16 everywhere in SBUF (x, weights, q/k/v1/pt/otp): halves DMA, enables
    FWL fast weight loads; PSUM accumulation stays f32.
  - x fully SBUF-resident (loaded once, 64 tiles of [128,512]).
  - projection order kv -> q0 -> q1 so attention(hp0) can start earlier.
  - scores per kt write a 2-bank [128,1024] f32 PSUM tile ([sA|sB]); one exp
    ACTIVATE of N=1024 -> pt2 [128,1024] bf16 (amortizes ACT's +352cyc/instr).
  - PSUM: 8 banks = scores 2x2 + pv 2 + shared acc/bps/yps 2 (+transp reuses pv).
  - y partials written bf16.
"""

import numpy as np

S = 2048
D = 2048
HD = 64
N_CORES = 8
ROPE_BASE = 10000.0

_cached = {}


def _build_program():
    import concourse.bass as bass
    import concourse.mybir as mybir
    import concourse.tile as tile
    from concourse import bacc

    BF16, F32 = mybir.dt.bfloat16, mybir.dt.float32
    EXP = mybir.ActivationFunctionType.Exp

    nc = bacc.Bacc("TRN2", target_bir_lowering=False, debug=False)

    xT = nc.dram_tensor("xT", [D, S], BF16, kind="ExternalInput").ap()
    wqt = nc.dram_tensor("wqt", [D, 256], BF16, kind="ExternalInput").ap()
    wkvt = nc.dram_tensor("wkvt", [D, 128], BF16, kind="ExternalInput").ap()
    wot = nc.dram_tensor("wot", [256, D], BF16, kind="ExternalInput").ap()
    cos2 = nc.dram_tensor("cos2", [128, S], F32, kind="ExternalInput").ap()
    sin2s = nc.dram_tensor("sin2s", [128, S], F32, kind="ExternalInput").ap()
    ones1 = nc.dram_tensor("ones1", [1, 64], BF16, kind="ExternalInput").ap()
    ident = nc.dram_tensor("ident", [64, 64], BF16, kind="ExternalInput").ap()
    y = nc.dram_tensor("y", [S, D], BF16, kind="ExternalOutput").ap()

    with tile.TileContext(nc) as tc:
        with tc.tile_pool(name="singles", bufs=1) as singles, \
             tc.tile_pool(name="rope", bufs=3) as rope, \
             tc.tile_pool(name="vtcp", bufs=2) as vtcp, \
             tc.tile_pool(name="ptp", bufs=3) as ptp, \
             tc.tile_pool(name="rcp", bufs=2) as rcp, \
             tc.tile_pool(name="bsbp", bufs=2) as bsbp, \
             tc.tile_pool(name="ysbp", bufs=3) as ysbp, \
             tc.tile_pool(name="pp", bufs=2, space="PSUM") as pp, \
             tc.tile_pool(name="pss", bufs=2, space="PSUM") as pss, \
             tc.tile_pool(name="pspv", bufs=2, space="PSUM") as pspv, \
             nc.allow_low_precision(reason="bf16 compute is intended"):

            # ---- static loads (order matters: earliest-needed first) ----
            wkv_t = []
            for k in range(16):
                t = singles.tile([128, 128], BF16, tag=f"wkv{k}", name=f"wkv{k}")
                nc.sync.dma_start(out=t, in_=wkvt[k * 128:(k + 1) * 128, :])
                wkv_t.append(t)
            cos_sb = singles.tile([128, S], F32, tag="cos")
            nc.sync.dma_start(out=cos_sb, in_=cos2)
            sin_sb = singles.tile([128, S], F32, tag="sin")
            nc.sync.dma_start(out=sin_sb, in_=sin2s)
            ident_sb = singles.tile([64, 64], BF16, tag="ident")
            nc.sync.dma_start(out=ident_sb, in_=ident)
            ones_sb = singles.tile([1, 64], BF16, tag="ones1")
            nc.sync.dma_start(out=ones_sb, in_=ones1)
            wq_t = []
            for k in range(16):
                t = singles.tile([128, 256], BF16, tag=f"wq{k}", name=f"wq{k}")
                nc.sync.dma_start(out=t, in_=wqt[k * 128:(k + 1) * 128, :])
                wq_t.append(t)
            # x resident: xs[k][sc] = xT[k*128:(k+1)*128, sc*512:(sc+1)*512]
            xs = [[None] * 4 for _ in range(16)]
            for sc in range(4):
                for k in range(16):
                    t = singles.tile([128, 512], BF16, tag=f"x{k}_{sc}",
                                     name=f"x{k}_{sc}")
                    nc.sync.dma_start(
                        out=t, in_=xT[k * 128:(k + 1) * 128,
                                      sc * 512:(sc + 1) * 512])
                    xs[k][sc] = t
            wo_t = []
            for i in range(2):
                t = singles.tile([128, 2048], BF16, tag=f"wo{i}", name=f"wo{i}")
                nc.sync.dma_start(out=t, in_=wot[i * 128:(i + 1) * 128, :])
                wo_t.append(t)

            qTr0 = singles.tile([128, S], BF16, tag="qTr0")
            qTr1 = singles.tile([128, S], BF16, tag="qTr1")
            kTr = singles.tile([128, S], BF16, tag="kTr")  # rows 64:128 dup 0:64
            otp0 = singles.tile([128, S], BF16, tag="otp0")
            otp1 = singles.tile([128, S], BF16, tag="otp1")
            v1 = [singles.tile([128, 65], BF16, tag=f"v1_{kt}", name=f"v1_{kt}")
                  for kt in range(16)]
            for kt in range(16):
                nc.vector.memset(v1[kt][:, 64:65], 1.0)

            # ---- phase 1: projections + RoPE + v transposes ----
            # ot order: kv first (attention needs full K/V), then q0, then q1.
            def rope_q(acc, dst, scs):
                t1 = rope.tile([128, 512], F32, tag="t1")
                t2 = rope.tile([128, 512], F32, tag="t2")
                nc.vector.tensor_mul(t1, acc, cos_sb[:, scs])
                nc.vector.tensor_mul(t2[0:32], acc[32:64], sin_sb[0:32, scs])
                nc.vector.tensor_mul(t2[32:64], acc[0:32], sin_sb[32:64, scs])
                nc.vector.tensor_mul(t2[64:96], acc[96:128], sin_sb[64:96, scs])
                nc.vector.tensor_mul(t2[96:128], acc[64:96], sin_sb[96:128, scs])
                nc.vector.tensor_add(dst[:, scs], t1, t2)

            for ot in (2, 0, 1):
                for sc in range(4):
                    scs = slice(sc * 512, (sc + 1) * 512)
                    acc = pp.tile([128, 512], F32, tag="acc")
                    for k in range(16):
                        lhsT = (wq_t[k][:, ot * 128:(ot + 1) * 128] if ot < 2
                                else wkv_t[k])
                        nc.tensor.matmul(acc, lhsT, xs[k][sc],
                                         start=(k == 0), stop=(k == 15))
                    if ot < 2:
                        rope_q(acc, qTr0 if ot == 0 else qTr1, scs)
                    else:
                        t1 = rope.tile([128, 512], F32, tag="t1")
                        t2 = rope.tile([128, 512], F32, tag="t2")
                        nc.vector.tensor_mul(t1[0:64], acc[0:64], cos_sb[0:64, scs])
                        nc.vector.tensor_mul(t2[0:32], acc[32:64], sin_sb[0:32, scs])
                        nc.vector.tensor_mul(t2[32:64], acc[0:32], sin_sb[32:64, scs])
                        nc.vector.tensor_add(kTr[0:64, scs], t1[0:64], t2[0:64])
                        nc.vector.tensor_copy(kTr[64:128, scs], kTr[0:64, scs])
                        vtc = vtcp.tile([64, 512], BF16, tag="vtc")
                        nc.vector.tensor_copy(vtc, acc[64:128])
                        for b in range(4):
                            kt = sc * 4 + b
                            tp = pspv.tile([128, 64], BF16, tag="pv",
                                           name=f"tp_{kt}")
                            nc.tensor.transpose(
                                tp, vtc[:, b * 128:(b + 1) * 128], ident_sb)
                            nc.vector.tensor_copy(v1[kt][:, 0:64], tp)

            # ---- phase 2: attention per 512-wide q chunk ----
            for qc in range(4):
                qs = slice(qc * 512, (qc + 1) * 512)
                for hp in range(2):
                    qsrc = qTr0 if hp == 0 else qTr1
                    dst = otp0 if hp == 0 else otp1
                    pvA = pspv.tile([65, 512], F32, tag="pv", name=f"pvA_{qc}_{hp}")
                    pvB = pspv.tile([65, 512], F32, tag="pv", name=f"pvB_{qc}_{hp}")
                    for kt in range(16):
                        ktc = slice(kt * 128, (kt + 1) * 128)
                        s2 = pss.tile([128, 1024], F32, tag="s",
                                      name=f"s_{qc}_{hp}_{kt}")
                        nc.tensor.matmul(s2[:, 0:512], kTr[0:64, ktc],
                                         qsrc[0:64, qs], start=True, stop=True)
                        nc.tensor.matmul(s2[:, 512:1024], kTr[64:128, ktc],
                                         qsrc[64:128, qs], start=True, stop=True)
                        pt2 = ptp.tile([128, 1024], BF16, tag="pt",
                                       name=f"pt_{qc}_{hp}_{kt}")
                        nc.scalar.activation(pt2, s2, EXP, scale=0.125)
                        nc.tensor.matmul(pvA, v1[kt], pt2[:, 0:512],
                                         start=(kt == 0), stop=(kt == 15))
                        nc.tensor.matmul(pvB, v1[kt], pt2[:, 512:1024],
                                         start=(kt == 0), stop=(kt == 15))
                    for sub, pv in ((0, pvA), (1, pvB)):
                        hrow = sub * 64
                        rc = rcp.tile([1, 512], BF16, tag="rc",
                                      name=f"rc_{qc}_{hp}_{sub}")
                        nc.vector.reciprocal(rc, pv[64:65, :])
                        bps = pp.tile([64, 512], F32, tag="acc",
                                      name=f"b_{qc}_{hp}_{sub}")
                        nc.tensor.matmul(bps, ones_sb, rc, start=True, stop=True)
                        bsb = bsbp.tile([64, 512], F32, tag="bsb",
                                        name=f"bsb_{qc}_{hp}_{sub}")
                        nc.vector.tensor_copy(bsb, bps)
                        nc.vector.tensor_mul(dst[hrow:hrow + 64, qs],
                                             pv[0:64, :], bsb)
                # ---- Wo for this q chunk (output rows qc*512..) ----
                for st in range(4):
                    sabs = qc * 4 + st
                    ss = slice(sabs * 128, (sabs + 1) * 128)
                    for mc in range(4):
                        ms = slice(mc * 512, (mc + 1) * 512)
                        yps = pp.tile([128, 512], F32, tag="acc",
                                      name=f"y_{sabs}_{mc}")
                        nc.tensor.matmul(yps, otp0[:, ss], wo_t[0][:, ms],
                                         start=True, stop=False)
                        nc.tensor.matmul(yps, otp1[:, ss], wo_t[1][:, ms],
                                         start=False, stop=True)
                        ysb = ysbp.tile([128, 512], BF16, tag="y")
                        nc.vector.tensor_copy(ysb, yps)
                        nc.gpsimd.dma_start(out=y[ss, ms], in_=ysb)

    nc.compile()
    return nc


def _host_prep(x, Wq, Wk, Wv, Wo):
    """Build per-core input maps (host-side numpy, untimed)."""
    import ml_dtypes
    bf16 = ml_dtypes.bfloat16

    x2 = np.ascontiguousarray(x.reshape(S, D), dtype=np.float32)
    xT = np.ascontiguousarray(x2.T).astype(bf16)

    inv = 1.0 / (ROPE_BASE ** (np.arange(0, HD, 2, dtype=np.float32) / HD))
    t = np.arange(S, dtype=np.float32)
    ang = np.einsum("i,j->ij", t, inv)              # [S, 32]
    emb = np.concatenate([ang, ang], axis=-1)       # [S, 64]
    cosT = np.ascontiguousarray(np.cos(emb).T.astype(np.float32))   # [64, S]
    sinT = np.ascontiguousarray(np.sin(emb).T.astype(np.float32))
    sinTs = sinT.copy()
    sinTs[0:32] *= -1.0
    cos2 = np.ascontiguousarray(np.concatenate([cosT, cosT], axis=0))
    sin2s = np.ascontiguousarray(np.concatenate([sinTs, sinTs], axis=0))

    ones1 = np.ones((1, 64), dtype=np.float32).astype(bf16)
    ident = np.eye(64, dtype=np.float32).astype(bf16)

    in_maps = []
    for c in range(N_CORES):
        osl = slice(c * 256, (c + 1) * 256)
        ksl = slice(c * 64, (c + 1) * 64)
        wqt = np.ascontiguousarray(Wq[osl, :].T).astype(bf16)          # [D, 256]
        wkvt = np.ascontiguousarray(
            np.concatenate([Wk[ksl, :], Wv[ksl, :]], axis=0).T).astype(bf16)
        wot = np.ascontiguousarray(Wo[:, osl].T).astype(bf16)          # [256, D]
        in_maps.append({
            "xT": xT, "wqt": wqt, "wkvt": wkvt, "wot": wot,
            "cos2": cos2, "sin2s": sin2s,
            "ones1": ones1, "ident": ident,
        })
    return in_maps


def kernel(x, Wq, Wk, Wv, Wo, _trace=False):
    from concourse.bass_utils import run_bass_kernel_spmd

    x = np.asarray(x, dtype=np.float32)
    Wq = np.asarray(Wq, dtype=np.float32)
    Wk = np.asarray(Wk, dtype=np.float32)
    Wv = np.asarray(Wv, dtype=np.float32)
    Wo = np.asarray(Wo, dtype=np.float32)

    if "nc" not in _cached:
        _cached["nc"] = _build_program()
    nc = _cached["nc"]

    in_maps = _host_prep(x, Wq, Wk, Wv, Wo)
    res = run_bass_kernel_spmd(nc, in_maps, core_ids=list(range(N_CORES)),
                               trace=_trace)
    out = np.zeros((S, D), dtype=np.float64)
    for r in res.results:
        out += np.asarray(r["y"]).astype(np.float64)
    _cached["last_results"] = res
    return out.astype(np.float32).reshape(1, S, D)


# revision 7
# speedup vs baseline: 1.1689x; 1.1689x over previous
"""GQA (B=1, S=2048, D=2048, 32 Q heads / 8 KV heads, head_dim=64, RoPE,
non-causal softmax) on 8 Trainium2 NeuronCores.

Sharding: tensor-parallel over heads. Core c owns Q heads 4c..4c+3 and KV head c.
Each core computes y_c = softmax(q_c k_c^T / 8) v_c @ Wo[:, c*256:(c+1)*256].T
(a full [S, D] partial, bf16); the host sums the 8 partials.

v3:
  - bf16 everywhere in SBUF; PSUM f32.
  - x loaded as 16 contiguous [128,2048] row-tiles (512KB/DMA); y stored as
    16 contiguous row-tiles.
  - attention kt-pair groups: 4 score MMs (64x128 tiling mode), 2 exps of
    N=1024, 4 PV MMs (128x128 mode) -> fewer array mode switches.
  - softmax bcast via [128,128] ones-row matmul (stays in 128x128 mode).
  - RoPE fully in bf16 on DVE; phase-1 copies on ScalarE (idle then).
  - filler queue: projection/Wo work is emitted in ~0.5-0.9us quanta pulled
    into the attention loop between kt groups, so the in-order PE stream has
    work while ACT (the exp bottleneck, ~147us) grinds.
"""

import numpy as np

S = 2048
D = 2048
HD = 64
N_CORES = 8
ROPE_BASE = 10000.0

_cached = {}


def _build_program():
    import concourse.bass as bass
    import concourse.mybir as mybir
    import concourse.tile as tile
    from concourse import bacc

    BF16, F32 = mybir.dt.bfloat16, mybir.dt.float32
    EXP = mybir.ActivationFunctionType.Exp

    nc = bacc.Bacc("TRN2", target_bir_lowering=False, debug=False)

    xT = nc.dram_tensor("xT", [D, S], BF16, kind="ExternalInput").ap()
    wqt = nc.dram_tensor("wqt", [D, 256], BF16, kind="ExternalInput").ap()
    wkvt = nc.dram_tensor("wkvt", [D, 128], BF16, kind="ExternalInput").ap()
    wot = nc.dram_tensor("wot", [256, D], BF16, kind="ExternalInput").ap()
    cos2 = nc.dram_tensor("cos2", [128, S], BF16, kind="ExternalInput").ap()
    sin2s = nc.dram_tensor("sin2s", [128, S], BF16, kind="ExternalInput").ap()
    onesr = nc.dram_tensor("onesr", [128, 128], BF16, kind="ExternalInput").ap()
    ident = nc.dram_tensor("ident", [64, 64], BF16, kind="ExternalInput").ap()
    y = nc.dram_tensor("y", [S, D], BF16, kind="ExternalOutput").ap()

    with tile.TileContext(nc) as tc:
        with tc.tile_pool(name="singles", bufs=1) as singles, \
             tc.tile_pool(name="rope", bufs=3) as rope, \
             tc.tile_pool(name="vtcp", bufs=2) as vtcp, \
             tc.tile_pool(name="ptp", bufs=4) as ptp, \
             tc.tile_pool(name="bsbp", bufs=2) as bsbp, \
             tc.tile_pool(name="ysbp", bufs=2) as ysbp, \
             tc.tile_pool(name="pp", bufs=2, space="PSUM") as pp, \
             tc.tile_pool(name="pss", bufs=2, space="PSUM") as pss, \
             tc.tile_pool(name="pspv", bufs=2, space="PSUM") as pspv, \
             nc.allow_low_precision(reason="bf16 compute is intended"):

            # ---- static loads (earliest-needed first) ----
            wkv_t = []
            for k in range(16):
                t = singles.tile([128, 128], BF16, tag=f"wkv{k}", name=f"wkv{k}")
                nc.sync.dma_start(out=t, in_=wkvt[k * 128:(k + 1) * 128, :])
                wkv_t.append(t)
            cos_sb = singles.tile([128, S], BF16, tag="cos")
            nc.sync.dma_start(out=cos_sb, in_=cos2)
            sin_sb = singles.tile([128, S], BF16, tag="sin")
            nc.sync.dma_start(out=sin_sb, in_=sin2s)
            ident_sb = singles.tile([64, 64], BF16, tag="ident")
            nc.sync.dma_start(out=ident_sb, in_=ident)
            onesr_sb = singles.tile([128, 128], BF16, tag="onesr")
            nc.sync.dma_start(out=onesr_sb, in_=onesr)
            # x resident as 16 full-row tiles (contiguous 512KB DMAs)
            xs = []
            for k in range(16):
                t = singles.tile([128, S], BF16, tag=f"x{k}", name=f"x{k}")
                nc.sync.dma_start(out=t, in_=xT[k * 128:(k + 1) * 128, :])
                xs.append(t)
            wq_t = []
            for k in range(16):
                t = singles.tile([128, 256], BF16, tag=f"wq{k}", name=f"wq{k}")
                nc.sync.dma_start(out=t, in_=wqt[k * 128:(k + 1) * 128, :])
                wq_t.append(t)
            wo_t = []
            for i in range(2):
                t = singles.tile([128, 2048], BF16, tag=f"wo{i}", name=f"wo{i}")
                nc.sync.dma_start(out=t, in_=wot[i * 128:(i + 1) * 128, :])
                wo_t.append(t)

            qTr0 = singles.tile([128, S], BF16, tag="qTr0")
            qTr1 = singles.tile([128, S], BF16, tag="qTr1")
            kTr = singles.tile([128, S], BF16, tag="kTr")  # rows 64:128 dup 0:64
            otp0 = singles.tile([128, S], BF16, tag="otp0")
            otp1 = singles.tile([128, S], BF16, tag="otp1")
            v1 = [singles.tile([128, 65], BF16, tag=f"v1_{kt}", name=f"v1_{kt}")
                  for kt in range(16)]
            for kt in range(16):
                nc.vector.memset(v1[kt][:, 64:65], 1.0)
            # reciprocal carriers: row 0 live, rows 1:128 stay zero forever
            rcb = []
            for i in range(2):
                t = singles.tile([128, 512], BF16, tag=f"rcb{i}", name=f"rcb{i}")
                nc.vector.memset(t, 0.0)
                rcb.append(t)

            def rope_q(acc, dst, scs, tagix):
                t0 = rope.tile([128, 512], BF16, tag="t0", name=f"t0_{tagix}")
                nc.vector.tensor_copy(t0, acc)  # psum f32 -> sbuf bf16
                t1 = rope.tile([128, 512], BF16, tag="t1", name=f"t1_{tagix}")
                t2 = rope.tile([128, 512], BF16, tag="t2", name=f"t2_{tagix}")
                # sin_sb rows are XOR-32 permuted on host: both TT inputs share
                # base partition (verifier NCC_IBIR297); output carries shift.
                nc.vector.tensor_mul(t1, t0, cos_sb[:, scs])
                nc.vector.tensor_mul(t2[0:32], t0[32:64], sin_sb[32:64, scs])
                nc.vector.tensor_mul(t2[32:64], t0[0:32], sin_sb[0:32, scs])
                nc.vector.tensor_mul(t2[64:96], t0[96:128], sin_sb[96:128, scs])
                nc.vector.tensor_mul(t2[96:128], t0[64:96], sin_sb[64:96, scs])
                nc.vector.tensor_add(dst[:, scs], t1, t2)

            def proj_kv(sc):
                scs = slice(sc * 512, (sc + 1) * 512)
                acc = pp.tile([128, 512], F32, tag="acc", name=f"acc_kv{sc}")
                for k in range(16):
                    nc.tensor.matmul(acc, wkv_t[k], xs[k][:, scs],
                                     start=(k == 0), stop=(k == 15))
                t0 = rope.tile([128, 512], BF16, tag="t0", name=f"t0_kv{sc}")
                nc.vector.tensor_copy(t0[0:64], acc[0:64])
                t1 = rope.tile([128, 512], BF16, tag="t1", name=f"t1_kv{sc}")
                t2 = rope.tile([128, 512], BF16, tag="t2", name=f"t2_kv{sc}")
                nc.vector.tensor_mul(t1[0:64], t0[0:64], cos_sb[0:64, scs])
                nc.vector.tensor_mul(t2[0:32], t0[32:64], sin_sb[32:64, scs])
                nc.vector.tensor_mul(t2[32:64], t0[0:32], sin_sb[0:32, scs])
                nc.vector.tensor_add(kTr[0:64, scs], t1[0:64], t2[0:64])
                nc.scalar.copy(kTr[64:128, scs], kTr[0:64, scs])
                vtc = vtcp.tile([64, 512], BF16, tag="vtc", name=f"vtc{sc}")
                nc.scalar.copy(vtc, acc[64:128])
                for b in range(4):
                    kt = sc * 4 + b
                    tp = pspv.tile([128, 64], BF16, tag="pv", name=f"tp_{kt}")
                    nc.tensor.transpose(tp, vtc[:, b * 128:(b + 1) * 128],
                                        ident_sb)
                    nc.scalar.copy(v1[kt][:, 0:64], tp)

            def proj_q(ot, sc):
                scs = slice(sc * 512, (sc + 1) * 512)
                acc = pp.tile([128, 512], F32, tag="acc", name=f"acc_{ot}_{sc}")
                for k in range(16):
                    nc.tensor.matmul(acc, wq_t[k][:, ot * 128:(ot + 1) * 128],
                                     xs[k][:, scs],
                                     start=(k == 0), stop=(k == 15))
                rope_q(acc, qTr0 if ot == 0 else qTr1, scs, f"{ot}_{sc}")

            # ---- filler queue: (cost_ns, closure) quanta of PE work ----
            filler = []

            def push_projq_quanta(ot, sc):
                scs = slice(sc * 512, (sc + 1) * 512)
                box = {}

                def mk(kq):
                    def run():
                        if kq == 0:
                            box["acc"] = pp.tile([128, 512], F32, tag="acc",
                                                 name=f"acc_{ot}_{sc}")
                        for k in range(kq * 4, kq * 4 + 4):
                            nc.tensor.matmul(
                                box["acc"],
                                wq_t[k][:, ot * 128:(ot + 1) * 128],
                                xs[k][:, scs],
                                start=(k == 0), stop=(k == 15))
                        if kq == 3:
                            rope_q(box["acc"], qTr0 if ot == 0 else qTr1,
                                   scs, f"{ot}_{sc}")
                    return run
                for kq in range(4):
                    filler.append((860, mk(kq)))

            def push_wo_quanta(qc):
                boxes = {}

                def mk(st, mc):
                    sabs = qc * 4 + st
                    ss = slice(sabs * 128, (sabs + 1) * 128)
                    ms = slice(mc * 512, (mc + 1) * 512)

                    def run():
                        if mc == 0:
                            boxes[st] = ysbp.tile([128, 2048], BF16, tag="y",
                                                  name=f"ysb_{sabs}")
                        yps = pp.tile([128, 512], F32, tag="acc",
                                      name=f"y_{sabs}_{mc}")
                        nc.tensor.matmul(yps, otp0[:, ss], wo_t[0][:, ms],
                                         start=True, stop=False)
                        nc.tensor.matmul(yps, otp1[:, ss], wo_t[1][:, ms],
                                         start=False, stop=True)
                        nc.vector.tensor_copy(boxes[st][:, ms], yps)
                        if mc == 3:
                            nc.gpsimd.dma_start(out=y[ss, :], in_=boxes[st])
                    return run
                for st in range(4):
                    for mc in range(4):
                        filler.append((480, mk(st, mc)))

            def pull(budget_ns):
                spent = 0
                while filler and spent < budget_ns:
                    cost, run = filler.pop(0)
                    run()
                    spent += cost

            # ---- attention for one (qc, hp) ----
            def attention(qc, hp):
                qs = slice(qc * 512, (qc + 1) * 512)
                qsrc = qTr0 if hp == 0 else qTr1
                dst = otp0 if hp == 0 else otp1
                pvA = pspv.tile([65, 512], F32, tag="pv", name=f"pvA_{qc}_{hp}")
                pvB = pspv.tile([65, 512], F32, tag="pv", name=f"pvB_{qc}_{hp}")
                pts = {}
                for g in range(8):  # kt pairs
                    s2s = []
                    for kt in (2 * g, 2 * g + 1):
                        ktc = slice(kt * 128, (kt + 1) * 128)
                        s2 = pss.tile([128, 1024], F32, tag="s",
                                      name=f"s_{qc}_{hp}_{kt}")
                        nc.tensor.matmul(s2[:, 0:512], kTr[0:64, ktc],
                                         qsrc[0:64, qs], start=True, stop=True)
                        nc.tensor.matmul(s2[:, 512:1024], kTr[64:128, ktc],
                                         qsrc[64:128, qs], start=True, stop=True)
                        s2s.append(s2)
                    for i, kt in enumerate((2 * g, 2 * g + 1)):
                        pt2 = ptp.tile([128, 1024], BF16, tag="pt",
                                       name=f"pt_{qc}_{hp}_{kt}")
                        nc.scalar.activation(pt2, s2s[i], EXP, scale=0.125)
                        pts[kt] = pt2
                    if g > 0:
                        for kt in (2 * g - 2, 2 * g - 1):
                            nc.tensor.matmul(pvA, v1[kt], pts[kt][:, 0:512],
                                             start=(kt == 0), stop=False)
                            nc.tensor.matmul(pvB, v1[kt], pts[kt][:, 512:1024],
                                             start=(kt == 0), stop=False)
                            del pts[kt]
                    pull(700)
                for kt in (14, 15):
                    nc.tensor.matmul(pvA, v1[kt], pts[kt][:, 0:512],
                                     start=False, stop=(kt == 15))
                    nc.tensor.matmul(pvB, v1[kt], pts[kt][:, 512:1024],
                                     start=False, stop=(kt == 15))
                    del pts[kt]
                for sub, pv in ((0, pvA), (1, pvB)):
                    hrow = sub * 64
                    nc.vector.reciprocal(rcb[sub][0:1, :], pv[64:65, :])
                    bps = pp.tile([128, 512], F32, tag="acc",
                                  name=f"b_{qc}_{hp}_{sub}")
                    nc.tensor.matmul(bps, onesr_sb, rcb[sub],
                                     start=True, stop=True)
                    bsb = bsbp.tile([64, 512], F32, tag="bsb",
                                    name=f"bsb_{qc}_{hp}_{sub}")
                    nc.vector.tensor_copy(bsb, bps[0:64])
                    nc.vector.tensor_mul(dst[hrow:hrow + 64, qs],
                                         pv[0:64, :], bsb)

            # ---- schedule ----
            for sc in range(4):
                proj_kv(sc)
            proj_q(0, 0)
            proj_q(1, 0)
            for qc in range(4):
                if qc < 3:
                    push_projq_quanta(0, qc + 1)
                    push_projq_quanta(1, qc + 1)
                attention(qc, 0)
                attention(qc, 1)
                push_wo_quanta(qc)
            pull(10**9)  # flush remaining Wo work

    nc.compile()
    return nc


def _host_prep(x, Wq, Wk, Wv, Wo):
    """Build per-core input maps (host-side numpy, untimed)."""
    import ml_dtypes
    bf16 = ml_dtypes.bfloat16

    x2 = np.ascontiguousarray(x.reshape(S, D), dtype=np.float32)
    xT = np.ascontiguousarray(x2.T).astype(bf16)

    inv = 1.0 / (ROPE_BASE ** (np.arange(0, HD, 2, dtype=np.float32) / HD))
    t = np.arange(S, dtype=np.float32)
    ang = np.einsum("i,j->ij", t, inv)              # [S, 32]
    emb = np.concatenate([ang, ang], axis=-1)       # [S, 64]
    cosT = np.ascontiguousarray(np.cos(emb).T.astype(np.float32))   # [64, S]
    sinT = np.ascontiguousarray(np.sin(emb).T.astype(np.float32))
    sinTs = sinT.copy()
    sinTs[0:32] *= -1.0
    cos2 = np.concatenate([cosT, cosT], axis=0).astype(bf16)
    sin2s_base = np.concatenate([sinTs, sinTs], axis=0)
    # XOR-32 row permutation: kernel reads sin at the *source* partition of
    # each rotate-half multiply so both DVE inputs share a base partition.
    perm = np.arange(128) ^ 32
    sin2s = np.ascontiguousarray(sin2s_base[perm]).astype(bf16)

    onesr = np.zeros((128, 128), dtype=np.float32)
    onesr[0, :] = 1.0
    onesr = onesr.astype(bf16)
    ident = np.eye(64, dtype=np.float32).astype(bf16)

    in_maps = []
    for c in range(N_CORES):
        osl = slice(c * 256, (c + 1) * 256)
        ksl = slice(c * 64, (c + 1) * 64)
        wqt = np.ascontiguousarray(Wq[osl, :].T).astype(bf16)          # [D, 256]
        wkvt = np.ascontiguousarray(
            np.concatenate([Wk[ksl, :], Wv[ksl, :]], axis=0).T).astype(bf16)
        wot = np.ascontiguousarray(Wo[:, osl].T).astype(bf16)          # [256, D]
        in_maps.append({
            "xT": xT, "wqt": wqt, "wkvt": wkvt, "wot": wot,
            "cos2": cos2, "sin2s": sin2s,
            "onesr": onesr, "ident": ident,
        })
    return in_maps


def kernel(x, Wq, Wk, Wv, Wo, _trace=False):
    from concourse.bass_utils import run_bass_kernel_spmd

    x = np.asarray(x, dtype=np.float32)
    Wq = np.asarray(Wq, dtype=np.float32)
    Wk = np.asarray(Wk, dtype=np.float32)
    Wv = np.asarray(Wv, dtype=np.float32)
    Wo = np.asarray(Wo, dtype=np.float32)

    if "nc" not in _cached:
        _cached["nc"] = _build_program()
    nc = _cached["nc"]

    in_maps = _host_prep(x, Wq, Wk, Wv, Wo)
    res = run_bass_kernel_spmd(nc, in_maps, core_ids=list(range(N_CORES)),
                               trace=_trace)
    out = np.zeros((S, D), dtype=np.float64)
    for r in res.results:
        out += np.asarray(r["y"]).astype(np.float64)
    _cached["last_results"] = res
    return out.astype(np.float32).reshape(1, S, D)


# revision 9
# speedup vs baseline: 1.1814x; 1.0107x over previous
"""GQA (B=1, S=2048, D=2048, 32 Q heads / 8 KV heads, head_dim=64, RoPE,
non-causal softmax) on 8 Trainium2 NeuronCores.

Sharding: tensor-parallel over heads. Core c owns Q heads 4c..4c+3 and KV head c.
Each core computes y_c = softmax(q_c k_c^T / 8) v_c @ Wo[:, c*256:(c+1)*256].T
(a full [S, D] partial, bf16); the host sums the 8 partials.

v4:
  - bf16 SBUF operands everywhere; PSUM f32.
  - input DMA split across sync/vector/scalar HWDGE rings; x as 16 contiguous
    [128,2048] row-tiles.
  - fused lead: all 4 KV chunks + q0/q1 chunk 0 accumulate in one k-loop while
    x streams in (KV accs live in the score-pool banks, q accs in the pv-pool
    banks -- both free during the lead), so compute tracks DMA arrival.
  - attention kt-pair groups: 4 score MMs (64x128 mode), 2 exps N=1024,
    4 PV MMs (128x128); softmax bcast via [128,128] ones-row matmul.
  - deferred norm: each attention's normalization is emitted two groups into
    the NEXT attention, so the boundary never starves ACT (the ~147us exp
    stream is the floor).
  - filler queue: remaining q projections + Wo in ~0.5-0.9us quanta pulled
    between kt groups; y stored as contiguous row-tiles on alternating rings.
"""

import numpy as np

S = 2048
D = 2048
HD = 64
N_CORES = 8
ROPE_BASE = 10000.0

_cached = {}


def _build_program():
    import concourse.bass as bass
    import concourse.mybir as mybir
    import concourse.tile as tile
    from concourse import bacc

    BF16, F32 = mybir.dt.bfloat16, mybir.dt.float32
    EXP = mybir.ActivationFunctionType.Exp

    nc = bacc.Bacc("TRN2", target_bir_lowering=False, debug=False)

    xT = nc.dram_tensor("xT", [D, S], BF16, kind="ExternalInput").ap()
    wqt = nc.dram_tensor("wqt", [D, 256], BF16, kind="ExternalInput").ap()
    wkvt = nc.dram_tensor("wkvt", [D, 128], BF16, kind="ExternalInput").ap()
    wot = nc.dram_tensor("wot", [256, D], BF16, kind="ExternalInput").ap()
    cos2 = nc.dram_tensor("cos2", [128, S], BF16, kind="ExternalInput").ap()
    sin2s = nc.dram_tensor("sin2s", [128, S], BF16, kind="ExternalInput").ap()
    onesr = nc.dram_tensor("onesr", [128, 128], BF16, kind="ExternalInput").ap()
    ident = nc.dram_tensor("ident", [64, 64], BF16, kind="ExternalInput").ap()
    y = nc.dram_tensor("y", [S, D], BF16, kind="ExternalOutput").ap()

    with tile.TileContext(nc) as tc:
        with tc.tile_pool(name="singles", bufs=1) as singles, \
             tc.tile_pool(name="rope", bufs=3) as rope, \
             tc.tile_pool(name="vtcp", bufs=2) as vtcp, \
             tc.tile_pool(name="ptp", bufs=4) as ptp, \
             tc.tile_pool(name="bsbp", bufs=2) as bsbp, \
             tc.tile_pool(name="ysbp", bufs=2) as ysbp, \
             tc.tile_pool(name="pp", bufs=2, space="PSUM") as pp, \
             tc.tile_pool(name="pss", bufs=2, space="PSUM") as pss, \
             tc.tile_pool(name="pspv", bufs=2, space="PSUM") as pspv, \
             nc.allow_low_precision(reason="bf16 compute is intended"):

            # ---- static loads: wkv+x feed the fused lead; rest on other rings
            wkv_t = []
            for k in range(16):
                t = singles.tile([128, 128], BF16, tag=f"wkv{k}", name=f"wkv{k}")
                nc.sync.dma_start(out=t, in_=wkvt[k * 128:(k + 1) * 128, :])
                wkv_t.append(t)
            # small weights needed at lead start: wq on the scalar HWDGE ring
            wq_t = []
            for k in range(16):
                t = singles.tile([128, 256], BF16, tag=f"wq{k}", name=f"wq{k}")
                nc.scalar.dma_start(out=t, in_=wqt[k * 128:(k + 1) * 128, :])
                wq_t.append(t)
            # tables / Wo on the gpsimd SWDGE ring (parallel, not latency-bound)
            cos_sb = singles.tile([128, S], BF16, tag="cos")
            nc.gpsimd.dma_start(out=cos_sb, in_=cos2)
            sin_sb = singles.tile([128, S], BF16, tag="sin")
            nc.gpsimd.dma_start(out=sin_sb, in_=sin2s)
            ident_sb = singles.tile([64, 64], BF16, tag="ident")
            nc.gpsimd.dma_start(out=ident_sb, in_=ident)
            onesr_sb = singles.tile([128, 128], BF16, tag="onesr")
            nc.gpsimd.dma_start(out=onesr_sb, in_=onesr)
            wo_t = []
            for i in range(2):
                t = singles.tile([128, 2048], BF16, tag=f"wo{i}", name=f"wo{i}")
                nc.gpsimd.dma_start(out=t, in_=wot[i * 128:(i + 1) * 128, :])
                wo_t.append(t)
            # x resident as 16 full-row tiles, alternating sync/scalar rings
            xs = []
            for k in range(16):
                t = singles.tile([128, S], BF16, tag=f"x{k}", name=f"x{k}")
                eng = nc.sync if k % 2 == 0 else nc.scalar
                eng.dma_start(out=t, in_=xT[k * 128:(k + 1) * 128, :])
                xs.append(t)

            qTr0 = singles.tile([128, S], BF16, tag="qTr0")
            qTr1 = singles.tile([128, S], BF16, tag="qTr1")
            kTr = singles.tile([128, S], BF16, tag="kTr")  # rows 64:128 dup 0:64
            otp0 = singles.tile([128, S], BF16, tag="otp0")
            otp1 = singles.tile([128, S], BF16, tag="otp1")
            v1 = [singles.tile([128, 65], BF16, tag=f"v1_{kt}", name=f"v1_{kt}")
                  for kt in range(16)]
            for kt in range(16):
                nc.vector.memset(v1[kt][:, 64:65], 1.0)
            rcb = []
            for i in range(2):
                t = singles.tile([128, 512], BF16, tag=f"rcb{i}", name=f"rcb{i}")
                nc.vector.memset(t, 0.0)
                rcb.append(t)

            def rope_q(acc, dst, scs, tagix):
                t0 = rope.tile([128, 512], BF16, tag="t0", name=f"t0_{tagix}")
                nc.vector.tensor_copy(t0, acc)  # psum f32 -> sbuf bf16
                t1 = rope.tile([128, 512], BF16, tag="t1", name=f"t1_{tagix}")
                t2 = rope.tile([128, 512], BF16, tag="t2", name=f"t2_{tagix}")
                # sin_sb rows are XOR-32 permuted on host: both TT inputs share
                # base partition (verifier NCC_IBIR297); output carries shift.
                nc.vector.tensor_mul(t1, t0, cos_sb[:, scs])
                nc.vector.tensor_mul(t2[0:32], t0[32:64], sin_sb[32:64, scs])
                nc.vector.tensor_mul(t2[32:64], t0[0:32], sin_sb[0:32, scs])
                nc.vector.tensor_mul(t2[64:96], t0[96:128], sin_sb[96:128, scs])
                nc.vector.tensor_mul(t2[96:128], t0[64:96], sin_sb[64:96, scs])
                nc.vector.tensor_add(dst[:, scs], t1, t2)

            def kv_post(acc, sc):
                """RoPE-K + kTr dup + v transposes for one 512-col kv chunk."""
                scs = slice(sc * 512, (sc + 1) * 512)
                t0 = rope.tile([128, 512], BF16, tag="t0", name=f"t0_kv{sc}")
                nc.vector.tensor_copy(t0[0:64], acc[0:64])
                t1 = rope.tile([128, 512], BF16, tag="t1", name=f"t1_kv{sc}")
                t2 = rope.tile([128, 512], BF16, tag="t2", name=f"t2_kv{sc}")
                nc.vector.tensor_mul(t1[0:64], t0[0:64], cos_sb[0:64, scs])
                nc.vector.tensor_mul(t2[0:32], t0[32:64], sin_sb[32:64, scs])
                nc.vector.tensor_mul(t2[32:64], t0[0:32], sin_sb[0:32, scs])
                nc.vector.tensor_add(kTr[0:64, scs], t1[0:64], t2[0:64])
                nc.scalar.copy(kTr[64:128, scs], kTr[0:64, scs])
                vtc = vtcp.tile([64, 512], BF16, tag="vtc", name=f"vtc{sc}")
                nc.scalar.copy(vtc, acc[64:128])
                for b in range(4):
                    kt = sc * 4 + b
                    tp = pspv.tile([128, 64], BF16, tag="pv", name=f"tp_{kt}")
                    nc.tensor.transpose(tp, vtc[:, b * 128:(b + 1) * 128],
                                        ident_sb)
                    nc.scalar.copy(v1[kt][:, 0:64], tp)

            # ---- fused lead: kv(sc0..3) + q0(sc0) + q1(sc0) in one k-loop ----
            skv = [pss.tile([128, 1024], F32, tag="s", name=f"skv{i}")
                   for i in range(2)]
            qacc = [pspv.tile([128, 512], F32, tag="pv", name=f"qacc{ot}")
                    for ot in range(2)]
            for k in range(16):
                st, sp = (k == 0), (k == 15)
                for sc in range(4):
                    nc.tensor.matmul(
                        skv[sc // 2][:, (sc % 2) * 512:(sc % 2) * 512 + 512],
                        wkv_t[k], xs[k][:, sc * 512:(sc + 1) * 512],
                        start=st, stop=sp)
                for ot in range(2):
                    nc.tensor.matmul(qacc[ot],
                                     wq_t[k][:, ot * 128:(ot + 1) * 128],
                                     xs[k][:, 0:512], start=st, stop=sp)
            rope_q(qacc[0], qTr0, slice(0, 512), "q0_0")
            rope_q(qacc[1], qTr1, slice(0, 512), "q1_0")
            for sc in range(4):
                kv_post(skv[sc // 2][:, (sc % 2) * 512:(sc % 2) * 512 + 512], sc)

            # ---- filler queue ----
            filler = []

            def push_projq_quanta(ot, sc):
                scs = slice(sc * 512, (sc + 1) * 512)
                box = {}

                def mk(kq):
                    def run():
                        if kq == 0:
                            box["acc"] = pp.tile([128, 512], F32, tag="acc",
                                                 name=f"acc_{ot}_{sc}")
                        for k in range(kq * 4, kq * 4 + 4):
                            nc.tensor.matmul(
                                box["acc"],
                                wq_t[k][:, ot * 128:(ot + 1) * 128],
                                xs[k][:, scs],
                                start=(k == 0), stop=(k == 15))
                        if kq == 3:
                            rope_q(box["acc"], qTr0 if ot == 0 else qTr1,
                                   scs, f"{ot}_{sc}")
                    return run
                for kq in range(4):
                    filler.append((860, mk(kq)))

            def push_wo_quanta(qc):
                boxes = {}

                def mk(st, mc):
                    sabs = qc * 4 + st
                    ss = slice(sabs * 128, (sabs + 1) * 128)
                    ms = slice(mc * 512, (mc + 1) * 512)

                    def run():
                        if mc == 0:
                            boxes[st] = ysbp.tile([128, 2048], BF16, tag="y",
                                                  name=f"ysb_{sabs}")
                        yps = pp.tile([128, 512], F32, tag="acc",
                                      name=f"y_{sabs}_{mc}")
                        nc.tensor.matmul(yps, otp0[:, ss], wo_t[0][:, ms],
                                         start=True, stop=False)
                        nc.tensor.matmul(yps, otp1[:, ss], wo_t[1][:, ms],
                                         start=False, stop=True)
                        nc.vector.tensor_copy(boxes[st][:, ms], yps)
                        if mc == 3:
                            eng = nc.gpsimd if sabs % 2 == 0 else nc.sync
                            eng.dma_start(out=y[ss, :], in_=boxes[st])
                    return run
                for st in range(4):
                    for mc in range(4):
                        filler.append((480, mk(st, mc)))

            def pull(budget_ns):
                spent = 0
                while filler and spent < budget_ns:
                    cost, run = filler.pop(0)
                    run()
                    spent += cost

            # ---- attention; norm is returned as a closure emitted later ----
            def attention(qc, hp, finish_prev=None, push_after=None):
                qs = slice(qc * 512, (qc + 1) * 512)
                qsrc = qTr0 if hp == 0 else qTr1
                dst = otp0 if hp == 0 else otp1
                pvA = pspv.tile([65, 512], F32, tag="pv", name=f"pvA_{qc}_{hp}")
                pvB = pspv.tile([65, 512], F32, tag="pv", name=f"pvB_{qc}_{hp}")
                pts = {}
                for g in range(8):  # kt pairs
                    s2s = []
                    for kt in (2 * g, 2 * g + 1):
                        ktc = slice(kt * 128, (kt + 1) * 128)
                        s2 = pss.tile([128, 1024], F32, tag="s",
                                      name=f"s_{qc}_{hp}_{kt}")
                        nc.tensor.matmul(s2[:, 0:512], kTr[0:64, ktc],
                                         qsrc[0:64, qs], start=True, stop=True)
                        nc.tensor.matmul(s2[:, 512:1024], kTr[64:128, ktc],
                                         qsrc[64:128, qs], start=True, stop=True)
                        s2s.append(s2)
                    for i, kt in enumerate((2 * g, 2 * g + 1)):
                        pt2 = ptp.tile([128, 1024], BF16, tag="pt",
                                       name=f"pt_{qc}_{hp}_{kt}")
                        nc.scalar.activation(pt2, s2s[i], EXP, scale=0.125)
                        pts[kt] = pt2
                    if g == 1:
                        if finish_prev is not None:
                            finish_prev()
                        if push_after is not None:
                            push_after()
                    if g > 0:
                        for kt in (2 * g - 2, 2 * g - 1):
                            nc.tensor.matmul(pvA, v1[kt], pts[kt][:, 0:512],
                                             start=(kt == 0), stop=False)
                            nc.tensor.matmul(pvB, v1[kt], pts[kt][:, 512:1024],
                                             start=(kt == 0), stop=False)
                            del pts[kt]
                    pull(700)
                for kt in (14, 15):
                    nc.tensor.matmul(pvA, v1[kt], pts[kt][:, 0:512],
                                     start=False, stop=(kt == 15))
                    nc.tensor.matmul(pvB, v1[kt], pts[kt][:, 512:1024],
                                     start=False, stop=(kt == 15))
                    del pts[kt]

                def finish():
                    for sub, pv in ((0, pvA), (1, pvB)):
                        hrow = sub * 64
                        nc.vector.reciprocal(rcb[sub][0:1, :], pv[64:65, :])
                        bps = pp.tile([128, 512], F32, tag="acc",
                                      name=f"b_{qc}_{hp}_{sub}")
                        nc.tensor.matmul(bps, onesr_sb, rcb[sub],
                                         start=True, stop=True)
                        bsb = bsbp.tile([64, 512], F32, tag="bsb",
                                        name=f"bsb_{qc}_{hp}_{sub}")
                        nc.vector.tensor_copy(bsb, bps[0:64])
                        nc.vector.tensor_mul(dst[hrow:hrow + 64, qs],
                                             pv[0:64, :], bsb)
                return finish

            # ---- schedule ----
            for sc in range(1, 4):
                push_projq_quanta(0, sc)
                push_projq_quanta(1, sc)
            fin = None
            for qc in range(4):
                pa = (lambda q=qc - 1: push_wo_quanta(q)) if qc > 0 else None
                fin = attention(qc, 0, fin, pa)
                fin = attention(qc, 1, fin, None)
            fin()
            push_wo_quanta(3)
            pull(10**9)  # flush remaining Wo work

    nc.compile()
    return nc


def _host_prep(x, Wq, Wk, Wv, Wo):
    """Build per-core input maps (host-side numpy, untimed)."""
    import ml_dtypes
    bf16 = ml_dtypes.bfloat16

    x2 = np.ascontiguousarray(x.reshape(S, D), dtype=np.float32)
    xT = np.ascontiguousarray(x2.T).astype(bf16)

    inv = 1.0 / (ROPE_BASE ** (np.arange(0, HD, 2, dtype=np.float32) / HD))
    t = np.arange(S, dtype=np.float32)
    ang = np.einsum("i,j->ij", t, inv)              # [S, 32]
    emb = np.concatenate([ang, ang], axis=-1)       # [S, 64]
    cosT = np.ascontiguousarray(np.cos(emb).T.astype(np.float32))   # [64, S]
    sinT = np.ascontiguousarray(np.sin(emb).T.astype(np.float32))
    sinTs = sinT.copy()
    sinTs[0:32] *= -1.0
    cos2 = np.concatenate([cosT, cosT], axis=0).astype(bf16)
    sin2s_base = np.concatenate([sinTs, sinTs], axis=0)
    # XOR-32 row permutation: kernel reads sin at the *source* partition of
    # each rotate-half multiply so both DVE inputs share a base partition.
    perm = np.arange(128) ^ 32
    sin2s = np.ascontiguousarray(sin2s_base[perm]).astype(bf16)

    onesr = np.zeros((128, 128), dtype=np.float32)
    onesr[0, :] = 1.0
    onesr = onesr.astype(bf16)
    ident = np.eye(64, dtype=np.float32).astype(bf16)

    in_maps = []
    for c in range(N_CORES):
        osl = slice(c * 256, (c + 1) * 256)
        ksl = slice(c * 64, (c + 1) * 64)
        wqt = np.ascontiguousarray(Wq[osl, :].T).astype(bf16)          # [D, 256]
        wkvt = np.ascontiguousarray(
            np.concatenate([Wk[ksl, :], Wv[ksl, :]], axis=0).T).astype(bf16)
        wot = np.ascontiguousarray(Wo[:, osl].T).astype(bf16)          # [256, D]
        in_maps.append({
            "xT": xT, "wqt": wqt, "wkvt": wkvt, "wot": wot,
            "cos2": cos2, "sin2s": sin2s,
            "onesr": onesr, "ident": ident,
        })
    return in_maps


def kernel(x, Wq, Wk, Wv, Wo, _trace=False):
    from concourse.bass_utils import run_bass_kernel_spmd

    x = np.asarray(x, dtype=np.float32)
    Wq = np.asarray(Wq, dtype=np.float32)
    Wk = np.asarray(Wk, dtype=np.float32)
    Wv = np.asarray(Wv, dtype=np.float32)
    Wo = np.asarray(Wo, dtype=np.float32)

    if "nc" not in _cached:
        _cached["nc"] = _build_program()
    nc = _cached["nc"]

    in_maps = _host_prep(x, Wq, Wk, Wv, Wo)
    res = run_bass_kernel_spmd(nc, in_maps, core_ids=list(range(N_CORES)),
                               trace=_trace)
    out = np.zeros((S, D), dtype=np.float64)
    for r in res.results:
        out += np.asarray(r["y"]).astype(np.float64)
    _cached["last_results"] = res
    return out.astype(np.float32).reshape(1, S, D)


# revision 13
# speedup vs baseline: 1.3430x; 1.1368x over previous
"""GQA (B=1, S=2048, D=2048, 32 Q heads / 8 KV heads, head_dim=64, RoPE,
non-causal softmax) on 8 Trainium2 NeuronCores.

Sharding: tensor-parallel over heads. Core c owns Q heads 4c..4c+3 and KV head c.
Each core computes y_c = softmax(q_c k_c^T / 8) v_c @ Wo[:, c*256:(c+1)*256].T
(a full [S, D] partial, bf16); the host sums the 8 partials.

v4:
  - bf16 SBUF operands everywhere; PSUM f32.
  - input DMA split across sync/vector/scalar HWDGE rings; x as 16 contiguous
    [128,2048] row-tiles.
  - fused lead: all 4 KV chunks + q0/q1 chunk 0 accumulate in one k-loop while
    x streams in (KV accs live in the score-pool banks, q accs in the pv-pool
    banks -- both free during the lead), so compute tracks DMA arrival.
  - attention kt-pair groups: 4 score MMs (64x128 mode), 2 exps N=1024,
    4 PV MMs (128x128); softmax bcast via [128,128] ones-row matmul.
  - deferred norm: each attention's normalization is emitted two groups into
    the NEXT attention, so the boundary never starves ACT (the ~147us exp
    stream is the floor).
  - filler queue: remaining q projections + Wo in ~0.5-0.9us quanta pulled
    between kt groups; y stored as contiguous row-tiles on alternating rings.
"""

import numpy as np

S = 2048
D = 2048
HD = 64
N_CORES = 8
ROPE_BASE = 10000.0

_cached = {}


def _build_program():
    import concourse.bass as bass
    import concourse.mybir as mybir
    import concourse.tile as tile
    from concourse import bacc

    BF16, F32 = mybir.dt.bfloat16, mybir.dt.float32
    F32R = mybir.dt.float32r
    EXP = mybir.ActivationFunctionType.Exp

    nc = bacc.Bacc("TRN2", target_bir_lowering=False, debug=False)

    xT = nc.dram_tensor("xT", [D, S], BF16, kind="ExternalInput").ap()
    wqkv = nc.dram_tensor("wqkv", [D, 384], BF16, kind="ExternalInput").ap()
    wot = nc.dram_tensor("wot", [256, D], BF16, kind="ExternalInput").ap()
    cos2 = nc.dram_tensor("cos2", [128, S], BF16, kind="ExternalInput").ap()
    sin2s = nc.dram_tensor("sin2s", [128, S], BF16, kind="ExternalInput").ap()
    onesr = nc.dram_tensor("onesr", [128, 128], BF16,
                           kind="ExternalInput").ap()
    ident = nc.dram_tensor("ident", [64, 64], BF16, kind="ExternalInput").ap()
    y = nc.dram_tensor("y", [S, D], BF16, kind="ExternalOutput").ap()

    with tile.TileContext(nc) as tc:
        with tc.tile_pool(name="singles", bufs=1) as singles, \
             tc.tile_pool(name="rope", bufs=3) as rope, \
             tc.tile_pool(name="vtcp", bufs=2) as vtcp, \
             tc.tile_pool(name="ptp", bufs=4) as ptp, \
             tc.tile_pool(name="bsbp", bufs=2) as bsbp, \
             tc.tile_pool(name="ysbp", bufs=2) as ysbp, \
             tc.tile_pool(name="pp", bufs=2, space="PSUM") as pp, \
             tc.tile_pool(name="pss", bufs=2, space="PSUM") as pss, \
             tc.tile_pool(name="pspv", bufs=2, space="PSUM") as pspv, \
             nc.allow_low_precision(reason="bf16 compute is intended"):

            # ---- static loads ----
            # wqkv[k] (768B rows, line-rate) interleaved with x[k] so the
            # fused lead can start as soon as the first pair lands; rings:
            #   sync:   wqkv/x even k        scalar: wqkv/x odd k, then wo
            #   gpsimd: cos/sin/ident/onesr (+ y stores later)
            cos_sb = singles.tile([128, S], BF16, tag="cos")
            nc.gpsimd.dma_start(out=cos_sb, in_=cos2)
            sin_sb = singles.tile([128, S], BF16, tag="sin")
            nc.gpsimd.dma_start(out=sin_sb, in_=sin2s)
            ident_sb = singles.tile([64, 64], BF16, tag="ident")
            nc.gpsimd.dma_start(out=ident_sb, in_=ident)
            onesr_sb = singles.tile([128, 128], BF16, tag="onesr")
            nc.gpsimd.dma_start(out=onesr_sb, in_=onesr)
            wqkv_t = []
            xs = []
            for k in range(16):
                eng = nc.sync if k % 2 == 0 else nc.scalar
                tw = singles.tile([128, 384], BF16, tag=f"wqkv{k}",
                                  name=f"wqkv{k}")
                eng.dma_start(out=tw, in_=wqkv[k * 128:(k + 1) * 128, :])
                wqkv_t.append(tw)
                tx = singles.tile([128, S], BF16, tag=f"x{k}", name=f"x{k}")
                eng.dma_start(out=tx, in_=xT[k * 128:(k + 1) * 128, :])
                xs.append(tx)
            wq_t = [t[:, 0:256] for t in wqkv_t]
            wkv_t = [t[:, 256:384] for t in wqkv_t]
            wo_t = []
            for i in range(2):
                t = singles.tile([128, 2048], BF16, tag=f"wo{i}", name=f"wo{i}")
                nc.scalar.dma_start(out=t, in_=wot[i * 128:(i + 1) * 128, :])
                wo_t.append(t)

            qTr0 = singles.tile([128, S], BF16, tag="qTr0")
            qTr1 = singles.tile([128, S], BF16, tag="qTr1")
            kTr = singles.tile([128, S], BF16, tag="kTr")  # rows 64:128 dup 0:64
            otp0 = singles.tile([128, S], BF16, tag="otp0")
            otp1 = singles.tile([128, S], BF16, tag="otp1")
            v1 = [singles.tile([128, 65], BF16, tag=f"v1_{kt}", name=f"v1_{kt}")
                  for kt in range(16)]
            for kt in range(16):
                nc.vector.memset(v1[kt][:, 64:65], 1.0)
            rcb = []
            rcs = []
            for i in range(2):
                t = singles.tile([128, 512], BF16, tag=f"rcb{i}", name=f"rcb{i}")
                nc.vector.memset(t, 0.0)
                rcb.append(t)
                t2_ = singles.tile([1, 512], F32, tag=f"rcs{i}", name=f"rcs{i}")
                rcs.append(t2_)
            lsb = [singles.tile([1, 512], F32, tag=f"lsb{i}", name=f"lsb{i}")
                   for i in range(2)]

            def rope_q(acc, dst, scs, tagix):
                t0 = rope.tile([128, 512], BF16, tag="t0", name=f"t0_{tagix}")
                nc.vector.tensor_copy(t0, acc)  # psum f32 -> sbuf bf16
                t1 = rope.tile([128, 512], BF16, tag="t1", name=f"t1_{tagix}")
                t2 = rope.tile([128, 512], BF16, tag="t2", name=f"t2_{tagix}")
                # sin_sb rows are XOR-32 permuted on host: both TT inputs share
                # base partition (verifier NCC_IBIR297); output carries shift.
                nc.vector.tensor_mul(t1, t0, cos_sb[:, scs])
                nc.vector.tensor_mul(t2[0:32], t0[32:64], sin_sb[32:64, scs])
                nc.vector.tensor_mul(t2[32:64], t0[0:32], sin_sb[0:32, scs])
                nc.vector.tensor_mul(t2[64:96], t0[96:128], sin_sb[96:128, scs])
                nc.vector.tensor_mul(t2[96:128], t0[64:96], sin_sb[64:96, scs])
                nc.vector.tensor_add(dst[:, scs], t1, t2)

            def kv_post(acc, sc):
                """RoPE-K + kTr dup + v transposes for one 512-col kv chunk."""
                scs = slice(sc * 512, (sc + 1) * 512)
                t0 = rope.tile([128, 512], BF16, tag="t0", name=f"t0_kv{sc}")
                nc.vector.tensor_copy(t0[0:64], acc[0:64])
                t1 = rope.tile([128, 512], BF16, tag="t1", name=f"t1_kv{sc}")
                t2 = rope.tile([128, 512], BF16, tag="t2", name=f"t2_kv{sc}")
                nc.vector.tensor_mul(t1[0:64], t0[0:64], cos_sb[0:64, scs])
                nc.vector.tensor_mul(t2[0:32], t0[32:64], sin_sb[32:64, scs])
                nc.vector.tensor_mul(t2[32:64], t0[0:32], sin_sb[0:32, scs])
                nc.vector.tensor_add(kTr[0:64, scs], t1[0:64], t2[0:64])
                nc.scalar.copy(kTr[64:128, scs], kTr[0:64, scs])
                vtc = vtcp.tile([64, 512], BF16, tag="vtc", name=f"vtc{sc}")
                nc.scalar.copy(vtc, acc[64:128])
                for b in range(4):
                    kt = sc * 4 + b
                    tp = pspv.tile([128, 64], BF16, tag="pv", name=f"tp_{kt}")
                    nc.tensor.transpose(tp, vtc[:, b * 128:(b + 1) * 128],
                                        ident_sb)
                    nc.scalar.copy(v1[kt][:, 0:64], tp)

            # ---- fused lead: kv(sc0..3) + q0(sc0) + q1(sc0) in one k-loop ----
            skv = [pss.tile([128, 1024], F32, tag="s", name=f"skv{i}")
                   for i in range(2)]
            qacc = [pspv.tile([128, 512], F32, tag="pv", name=f"qacc{ot}")
                    for ot in range(2)]
            for k in range(16):
                st, sp = (k == 0), (k == 15)
                for sc in range(4):
                    nc.tensor.matmul(
                        skv[sc // 2][:, (sc % 2) * 512:(sc % 2) * 512 + 512],
                        wkv_t[k], xs[k][:, sc * 512:(sc + 1) * 512],
                        start=st, stop=sp)
                for ot in range(2):
                    nc.tensor.matmul(qacc[ot],
                                     wq_t[k][:, ot * 128:(ot + 1) * 128],
                                     xs[k][:, 0:512], start=st, stop=sp)
            rope_q(qacc[0], qTr0, slice(0, 512), "q0_0")
            rope_q(qacc[1], qTr1, slice(0, 512), "q1_0")
            for sc in range(4):
                kv_post(skv[sc // 2][:, (sc % 2) * 512:(sc % 2) * 512 + 512], sc)

            # ---- filler queue ----
            filler = []

            def push_projq_quanta(ot, sc):
                scs = slice(sc * 512, (sc + 1) * 512)
                box = {}

                def mk(kq):
                    def run():
                        if kq == 0:
                            box["acc"] = pp.tile([128, 512], F32, tag="acc",
                                                 name=f"acc_{ot}_{sc}")
                        for k in range(kq * 2, kq * 2 + 2):
                            nc.tensor.matmul(
                                box["acc"],
                                wq_t[k][:, ot * 128:(ot + 1) * 128],
                                xs[k][:, scs],
                                start=(k == 0), stop=(k == 15))
                        if kq == 7:
                            rope_q(box["acc"], qTr0 if ot == 0 else qTr1,
                                   scs, f"{ot}_{sc}")
                    return run
                for kq in range(8):
                    filler.append((430, mk(kq)))

            def push_wo_quanta(qc):
                boxes = {}

                def mk(st, mc):
                    sabs = qc * 4 + st
                    ss = slice(sabs * 128, (sabs + 1) * 128)
                    ms = slice(mc * 512, (mc + 1) * 512)

                    def run():
                        if mc == 0:
                            boxes[st] = ysbp.tile([128, 2048], BF16, tag="y",
                                                  name=f"ysb_{sabs}")
                        yps = pp.tile([128, 512], F32, tag="acc",
                                      name=f"y_{sabs}_{mc}")
                        nc.tensor.matmul(yps, otp0[:, ss], wo_t[0][:, ms],
                                         start=True, stop=False)
                        nc.tensor.matmul(yps, otp1[:, ss], wo_t[1][:, ms],
                                         start=False, stop=True)
                        nc.vector.tensor_copy(boxes[st][:, ms], yps)
                        if mc == 3:
                            eng = nc.gpsimd if sabs % 2 == 0 else nc.sync
                            eng.dma_start(out=y[ss, :], in_=boxes[st])
                    return run
                for st in range(4):
                    for mc in range(4):
                        filler.append((480, mk(st, mc)))

            def pull(budget_ns):
                spent = 0
                while filler and spent < budget_ns:
                    cost, run = filler.pop(0)
                    run()
                    spent += cost

            # ---- attention; norm is returned as a closure emitted later ----
            def attention(qc, hp, finish_prev=None, push_after=None):
                qs = slice(qc * 512, (qc + 1) * 512)
                qsrc = qTr0 if hp == 0 else qTr1
                dst = otp0 if hp == 0 else otp1
                pvA = pspv.tile([65, 512], F32, tag="pv", name=f"pvA_{qc}_{hp}")
                pvB = pspv.tile([65, 512], F32, tag="pv", name=f"pvB_{qc}_{hp}")
                pts = {}
                for g in range(8):  # kt pairs
                    s2s = []
                    for kt in (2 * g, 2 * g + 1):
                        ktc = slice(kt * 128, (kt + 1) * 128)
                        s2 = pss.tile([128, 1024], F32, tag="s",
                                      name=f"s_{qc}_{hp}_{kt}")
                        nc.tensor.matmul(s2[:, 0:512], kTr[0:64, ktc],
                                         qsrc[0:64, qs], start=True, stop=True)
                        nc.tensor.matmul(s2[:, 512:1024], kTr[64:128, ktc],
                                         qsrc[64:128, qs], start=True, stop=True)
                        s2s.append(s2)
                    for i, kt in enumerate((2 * g, 2 * g + 1)):
                        pt2 = ptp.tile([128, 1024], BF16, tag="pt",
                                       name=f"pt_{qc}_{hp}_{kt}")
                        nc.scalar.activation(pt2, s2s[i], EXP, scale=0.125)
                        pts[kt] = pt2
                    if g == 1:
                        if finish_prev is not None:
                            finish_prev()
                        if push_after is not None:
                            push_after()
                    if g > 0:
                        for kt in (2 * g - 2, 2 * g - 1):
                            nc.tensor.matmul(pvA, v1[kt], pts[kt][:, 0:512],
                                             start=(kt == 0), stop=False)
                            nc.tensor.matmul(pvB, v1[kt], pts[kt][:, 512:1024],
                                             start=(kt == 0), stop=False)
                            del pts[kt]
                    pull(1000)
                for kt in (14, 15):
                    nc.tensor.matmul(pvA, v1[kt], pts[kt][:, 0:512],
                                     start=False, stop=(kt == 15))
                    nc.tensor.matmul(pvB, v1[kt], pts[kt][:, 512:1024],
                                     start=False, stop=(kt == 15))
                    del pts[kt]

                def finish():
                    # custom-DVE recip reads garbage from PSUM: stage via SBUF
                    for sub, pv in ((0, pvA), (1, pvB)):
                        nc.vector.tensor_copy(lsb[sub], pv[64:65, :])
                    for sub in (0, 1):
                        nc.vector.reciprocal_approx_fast(rcs[sub], lsb[sub])
                    for sub in (0, 1):
                        nc.vector.tensor_copy(rcb[sub][0:1, :], rcs[sub])
                    bpss, bsbs = [], []
                    for sub in (0, 1):
                        bps = pp.tile([128, 512], F32, tag="acc",
                                      name=f"b_{qc}_{hp}_{sub}")
                        nc.tensor.matmul(bps, onesr_sb, rcb[sub],
                                         start=True, stop=True)
                        bpss.append(bps)
                    for sub in (0, 1):
                        bsb = bsbp.tile([64, 512], F32, tag="bsb",
                                        name=f"bsb_{qc}_{hp}_{sub}")
                        nc.vector.tensor_copy(bsb, bpss[sub][0:64])
                        bsbs.append(bsb)
                    for sub, pv in ((0, pvA), (1, pvB)):
                        nc.vector.tensor_mul(dst[sub * 64:sub * 64 + 64, qs],
                                             pv[0:64, :], bsbs[sub])
                return finish

            # ---- schedule ----
            for sc in range(1, 4):
                push_projq_quanta(0, sc)
                push_projq_quanta(1, sc)
            fin = None
            for qc in range(4):
                pa = (lambda q=qc - 1: push_wo_quanta(q)) if qc > 0 else None
                fin = attention(qc, 0, fin, pa)
                fin = attention(qc, 1, fin, None)
            fin()
            push_wo_quanta(3)
            pull(10**9)  # flush remaining Wo work

    nc.compile()
    return nc


def _host_prep(x, Wq, Wk, Wv, Wo):
    """Build per-core input maps (host-side numpy, untimed)."""
    import ml_dtypes
    bf16 = ml_dtypes.bfloat16

    x2 = np.ascontiguousarray(x.reshape(S, D), dtype=np.float32)
    xT = np.ascontiguousarray(x2.T).astype(bf16)

    inv = 1.0 / (ROPE_BASE ** (np.arange(0, HD, 2, dtype=np.float32) / HD))
    t = np.arange(S, dtype=np.float32)
    ang = np.einsum("i,j->ij", t, inv)              # [S, 32]
    emb = np.concatenate([ang, ang], axis=-1)       # [S, 64]
    cosT = np.ascontiguousarray(np.cos(emb).T.astype(np.float32))   # [64, S]
    sinT = np.ascontiguousarray(np.sin(emb).T.astype(np.float32))
    sinTs = sinT.copy()
    sinTs[0:32] *= -1.0
    cos2 = np.concatenate([cosT, cosT], axis=0).astype(bf16)
    sin2s_base = np.concatenate([sinTs, sinTs], axis=0)
    # XOR-32 row permutation: kernel reads sin at the *source* partition of
    # each rotate-half multiply so both DVE inputs share a base partition.
    perm = np.arange(128) ^ 32
    sin2s = np.ascontiguousarray(sin2s_base[perm]).astype(bf16)

    onesr = np.zeros((128, 128), dtype=np.float32)
    onesr[0, :] = 1.0
    onesr = onesr.astype(bf16)
    ident = np.eye(64, dtype=np.float32).astype(bf16)

    in_maps = []
    for c in range(N_CORES):
        osl = slice(c * 256, (c + 1) * 256)
        ksl = slice(c * 64, (c + 1) * 64)
        # packed [D, 384] = [Wq_c^T (256) | Wk_c^T (64) | Wv_c^T (64)]
        wqkv = np.ascontiguousarray(np.concatenate(
            [Wq[osl, :], Wk[ksl, :], Wv[ksl, :]], axis=0).T).astype(bf16)
        wot = np.ascontiguousarray(Wo[:, osl].T).astype(bf16)          # [256, D]
        in_maps.append({
            "xT": xT, "wqkv": wqkv, "wot": wot,
            "cos2": cos2, "sin2s": sin2s,
            "onesr": onesr, "ident": ident,
        })
    return in_maps


def kernel(x, Wq, Wk, Wv, Wo, _trace=False):
    from concourse.bass_utils import run_bass_kernel_spmd

    x = np.asarray(x, dtype=np.float32)
    Wq = np.asarray(Wq, dtype=np.float32)
    Wk = np.asarray(Wk, dtype=np.float32)
    Wv = np.asarray(Wv, dtype=np.float32)
    Wo = np.asarray(Wo, dtype=np.float32)

    if "nc" not in _cached:
        _cached["nc"] = _build_program()
    nc = _cached["nc"]

    in_maps = _host_prep(x, Wq, Wk, Wv, Wo)
    res = run_bass_kernel_spmd(nc, in_maps, core_ids=list(range(N_CORES)),
                               trace=_trace)
    out = np.zeros((S, D), dtype=np.float64)
    for r in res.results:
        out += np.asarray(r["y"]).astype(np.float64)
    _cached["last_results"] = res
    return out.astype(np.float32).reshape(1, S, D)


# revision 14
# speedup vs baseline: 1.3644x; 1.0159x over previous
"""GQA (B=1, S=2048, D=2048, 32 Q heads / 8 KV heads, head_dim=64, RoPE,
non-causal softmax) on 8 Trainium2 NeuronCores.

Sharding: tensor-parallel over heads. Core c owns Q heads 4c..4c+3 and KV head c.
Each core computes y_c = softmax(q_c k_c^T / 8) v_c @ Wo[:, c*256:(c+1)*256].T
(a full [S, D] partial, bf16); the host sums the 8 partials.

v4:
  - bf16 SBUF operands everywhere; PSUM f32.
  - input DMA split across sync/vector/scalar HWDGE rings; x as 16 contiguous
    [128,2048] row-tiles.
  - fused lead: all 4 KV chunks + q0/q1 chunk 0 accumulate in one k-loop while
    x streams in (KV accs live in the score-pool banks, q accs in the pv-pool
    banks -- both free during the lead), so compute tracks DMA arrival.
  - attention kt-pair groups: 4 score MMs (64x128 mode), 2 exps N=1024,
    4 PV MMs (128x128); softmax bcast via [128,128] ones-row matmul.
  - deferred norm: each attention's normalization is emitted two groups into
    the NEXT attention, so the boundary never starves ACT (the ~147us exp
    stream is the floor).
  - filler queue: remaining q projections + Wo in ~0.5-0.9us quanta pulled
    between kt groups; y stored as contiguous row-tiles on alternating rings.
"""

import numpy as np

S = 2048
D = 2048
HD = 64
N_CORES = 8
ROPE_BASE = 10000.0

_cached = {}


def _build_program():
    import concourse.bass as bass
    import concourse.mybir as mybir
    import concourse.tile as tile
    from concourse import bacc

    BF16, F32 = mybir.dt.bfloat16, mybir.dt.float32
    F32R = mybir.dt.float32r
    EXP = mybir.ActivationFunctionType.Exp

    nc = bacc.Bacc("TRN2", target_bir_lowering=False, debug=False)

    xT = nc.dram_tensor("xT", [D, S], BF16, kind="ExternalInput").ap()
    wqkv = nc.dram_tensor("wqkv", [D, 384], BF16, kind="ExternalInput").ap()
    wot = nc.dram_tensor("wot", [256, D], BF16, kind="ExternalInput").ap()
    cos2 = nc.dram_tensor("cos2", [128, S], BF16, kind="ExternalInput").ap()
    sin2s = nc.dram_tensor("sin2s", [128, S], BF16, kind="ExternalInput").ap()
    onesr = nc.dram_tensor("onesr", [128, 128], BF16,
                           kind="ExternalInput").ap()
    ident = nc.dram_tensor("ident", [64, 64], BF16, kind="ExternalInput").ap()
    y = nc.dram_tensor("y", [S, D], BF16, kind="ExternalOutput").ap()

    with tile.TileContext(nc) as tc:
        with tc.tile_pool(name="singles", bufs=1) as singles, \
             tc.tile_pool(name="rope", bufs=3) as rope, \
             tc.tile_pool(name="vtcp", bufs=2) as vtcp, \
             tc.tile_pool(name="ptp", bufs=6) as ptp, \
             tc.tile_pool(name="bsbp", bufs=2) as bsbp, \
             tc.tile_pool(name="ysbp", bufs=2) as ysbp, \
             tc.tile_pool(name="pp", bufs=2, space="PSUM") as pp, \
             tc.tile_pool(name="pss", bufs=2, space="PSUM") as pss, \
             tc.tile_pool(name="pspv", bufs=2, space="PSUM") as pspv, \
             nc.allow_low_precision(reason="bf16 compute is intended"):

            # ---- static loads ----
            # wqkv[k] (768B rows, line-rate) interleaved with x[k] so the
            # fused lead can start as soon as the first pair lands; rings:
            #   sync:   wqkv/x even k        scalar: wqkv/x odd k, then wo
            #   gpsimd: cos/sin/ident/onesr (+ y stores later)
            cos_sb = singles.tile([128, S], BF16, tag="cos")
            nc.gpsimd.dma_start(out=cos_sb, in_=cos2)
            sin_sb = singles.tile([128, S], BF16, tag="sin")
            nc.gpsimd.dma_start(out=sin_sb, in_=sin2s)
            ident_sb = singles.tile([64, 64], BF16, tag="ident")
            nc.gpsimd.dma_start(out=ident_sb, in_=ident)
            onesr_sb = singles.tile([128, 128], BF16, tag="onesr")
            nc.gpsimd.dma_start(out=onesr_sb, in_=onesr)
            wqkv_t = []
            xs = []
            for k in range(16):
                eng = nc.sync if k % 2 == 0 else nc.scalar
                tw = singles.tile([128, 384], BF16, tag=f"wqkv{k}",
                                  name=f"wqkv{k}")
                eng.dma_start(out=tw, in_=wqkv[k * 128:(k + 1) * 128, :])
                wqkv_t.append(tw)
                tx = singles.tile([128, S], BF16, tag=f"x{k}", name=f"x{k}")
                eng.dma_start(out=tx, in_=xT[k * 128:(k + 1) * 128, :])
                xs.append(tx)
            wq_t = [t[:, 0:256] for t in wqkv_t]
            wkv_t = [t[:, 256:384] for t in wqkv_t]
            wo_t = []
            for i in range(2):
                t = singles.tile([128, 2048], BF16, tag=f"wo{i}", name=f"wo{i}")
                nc.scalar.dma_start(out=t, in_=wot[i * 128:(i + 1) * 128, :])
                wo_t.append(t)

            qTr0 = singles.tile([128, S], BF16, tag="qTr0")
            qTr1 = singles.tile([128, S], BF16, tag="qTr1")
            kTr = singles.tile([128, S], BF16, tag="kTr")  # rows 64:128 dup 0:64
            otp0 = singles.tile([128, S], BF16, tag="otp0")
            otp1 = singles.tile([128, S], BF16, tag="otp1")
            v1 = [singles.tile([128, 65], BF16, tag=f"v1_{kt}", name=f"v1_{kt}")
                  for kt in range(16)]
            for kt in range(16):
                nc.vector.memset(v1[kt][:, 64:65], 1.0)
            rcb = []
            rcs = []
            for i in range(2):
                t = singles.tile([128, 512], BF16, tag=f"rcb{i}", name=f"rcb{i}")
                nc.vector.memset(t, 0.0)
                rcb.append(t)
                t2_ = singles.tile([1, 512], F32, tag=f"rcs{i}", name=f"rcs{i}")
                rcs.append(t2_)
            lsb = [singles.tile([1, 512], F32, tag=f"lsb{i}", name=f"lsb{i}")
                   for i in range(2)]

            def rope_q(acc, dst, scs, tagix):
                t0 = rope.tile([128, 512], BF16, tag="t0", name=f"t0_{tagix}")
                nc.vector.tensor_copy(t0, acc)  # psum f32 -> sbuf bf16
                t1 = rope.tile([128, 512], BF16, tag="t1", name=f"t1_{tagix}")
                t2 = rope.tile([128, 512], BF16, tag="t2", name=f"t2_{tagix}")
                # sin_sb rows are XOR-32 permuted on host: both TT inputs share
                # base partition (verifier NCC_IBIR297); output carries shift.
                nc.vector.tensor_mul(t1, t0, cos_sb[:, scs])
                nc.vector.tensor_mul(t2[0:32], t0[32:64], sin_sb[32:64, scs])
                nc.vector.tensor_mul(t2[32:64], t0[0:32], sin_sb[0:32, scs])
                nc.vector.tensor_mul(t2[64:96], t0[96:128], sin_sb[96:128, scs])
                nc.vector.tensor_mul(t2[96:128], t0[64:96], sin_sb[64:96, scs])
                nc.vector.tensor_add(dst[:, scs], t1, t2)

            def kv_post(acc, sc):
                """RoPE-K + kTr dup + v transposes for one 512-col kv chunk."""
                scs = slice(sc * 512, (sc + 1) * 512)
                t0 = rope.tile([128, 512], BF16, tag="t0", name=f"t0_kv{sc}")
                nc.vector.tensor_copy(t0[0:64], acc[0:64])
                t1 = rope.tile([128, 512], BF16, tag="t1", name=f"t1_kv{sc}")
                t2 = rope.tile([128, 512], BF16, tag="t2", name=f"t2_kv{sc}")
                nc.vector.tensor_mul(t1[0:64], t0[0:64], cos_sb[0:64, scs])
                nc.vector.tensor_mul(t2[0:32], t0[32:64], sin_sb[32:64, scs])
                nc.vector.tensor_mul(t2[32:64], t0[0:32], sin_sb[0:32, scs])
                nc.vector.tensor_add(kTr[0:64, scs], t1[0:64], t2[0:64])
                nc.scalar.copy(kTr[64:128, scs], kTr[0:64, scs])
                vtc = vtcp.tile([64, 512], BF16, tag="vtc", name=f"vtc{sc}")
                nc.scalar.copy(vtc, acc[64:128])
                for b in range(4):
                    kt = sc * 4 + b
                    tp = pspv.tile([128, 64], BF16, tag="pv", name=f"tp_{kt}")
                    nc.tensor.transpose(tp, vtc[:, b * 128:(b + 1) * 128],
                                        ident_sb)
                    nc.scalar.copy(v1[kt][:, 0:64], tp)

            # ---- fused lead: kv(sc0..3) + q0(sc0) + q1(sc0) in one k-loop ----
            skv = [pss.tile([128, 1024], F32, tag="s", name=f"skv{i}")
                   for i in range(2)]
            qacc = [pspv.tile([128, 512], F32, tag="pv", name=f"qacc{ot}")
                    for ot in range(2)]
            for k in range(16):
                st, sp = (k == 0), (k == 15)
                for sc in range(4):
                    nc.tensor.matmul(
                        skv[sc // 2][:, (sc % 2) * 512:(sc % 2) * 512 + 512],
                        wkv_t[k], xs[k][:, sc * 512:(sc + 1) * 512],
                        start=st, stop=sp)
                for ot in range(2):
                    nc.tensor.matmul(qacc[ot],
                                     wq_t[k][:, ot * 128:(ot + 1) * 128],
                                     xs[k][:, 0:512], start=st, stop=sp)
            rope_q(qacc[0], qTr0, slice(0, 512), "q0_0")
            rope_q(qacc[1], qTr1, slice(0, 512), "q1_0")
            for sc in range(4):
                kv_post(skv[sc // 2][:, (sc % 2) * 512:(sc % 2) * 512 + 512], sc)

            # ---- filler queue ----
            filler = []

            def push_projq_quanta(ot, sc):
                scs = slice(sc * 512, (sc + 1) * 512)
                box = {}

                def mk(kq):
                    def run():
                        if kq == 0:
                            box["acc"] = pp.tile([128, 512], F32, tag="acc",
                                                 name=f"acc_{ot}_{sc}")
                        for k in range(kq * 2, kq * 2 + 2):
                            nc.tensor.matmul(
                                box["acc"],
                                wq_t[k][:, ot * 128:(ot + 1) * 128],
                                xs[k][:, scs],
                                start=(k == 0), stop=(k == 15))
                        if kq == 7:
                            rope_q(box["acc"], qTr0 if ot == 0 else qTr1,
                                   scs, f"{ot}_{sc}")
                    return run
                for kq in range(8):
                    filler.append((430, mk(kq)))

            def push_wo_quanta(qc):
                boxes = {}

                def mk(st, mc):
                    sabs = qc * 4 + st
                    ss = slice(sabs * 128, (sabs + 1) * 128)
                    ms = slice(mc * 512, (mc + 1) * 512)

                    def run():
                        if mc == 0:
                            boxes[st] = ysbp.tile([128, 2048], BF16, tag="y",
                                                  name=f"ysb_{sabs}")
                        yps = pp.tile([128, 512], F32, tag="acc",
                                      name=f"y_{sabs}_{mc}")
                        nc.tensor.matmul(yps, otp0[:, ss], wo_t[0][:, ms],
                                         start=True, stop=False)
                        nc.tensor.matmul(yps, otp1[:, ss], wo_t[1][:, ms],
                                         start=False, stop=True)
                        if qc == 3:
                            nc.scalar.copy(boxes[st][:, ms], yps)
                        else:
                            nc.vector.tensor_copy(boxes[st][:, ms], yps)
                        if mc == 3:
                            eng = nc.gpsimd if sabs % 2 == 0 else nc.sync
                            eng.dma_start(out=y[ss, :], in_=boxes[st])
                    return run
                for st in range(4):
                    for mc in range(4):
                        filler.append((480, mk(st, mc)))

            def pull(budget_ns):
                spent = 0
                while filler and spent < budget_ns:
                    cost, run = filler.pop(0)
                    run()
                    spent += cost

            # ---- attention; norm is returned as a closure emitted later ----
            def attention(qc, hp, finish_prev=None, push_after=None):
                qs = slice(qc * 512, (qc + 1) * 512)
                qsrc = qTr0 if hp == 0 else qTr1
                dst = otp0 if hp == 0 else otp1
                pvA = pspv.tile([65, 512], F32, tag="pv", name=f"pvA_{qc}_{hp}")
                pvB = pspv.tile([65, 512], F32, tag="pv", name=f"pvB_{qc}_{hp}")
                pts = {}
                for g in range(8):  # kt pairs
                    s2s = []
                    for kt in (2 * g, 2 * g + 1):
                        ktc = slice(kt * 128, (kt + 1) * 128)
                        s2 = pss.tile([128, 1024], F32, tag="s",
                                      name=f"s_{qc}_{hp}_{kt}")
                        nc.tensor.matmul(s2[:, 0:512], kTr[0:64, ktc],
                                         qsrc[0:64, qs], start=True, stop=True)
                        nc.tensor.matmul(s2[:, 512:1024], kTr[64:128, ktc],
                                         qsrc[64:128, qs], start=True, stop=True)
                        s2s.append(s2)
                    for i, kt in enumerate((2 * g, 2 * g + 1)):
                        pt2 = ptp.tile([128, 1024], BF16, tag="pt",
                                       name=f"pt_{qc}_{hp}_{kt}")
                        nc.scalar.activation(pt2, s2s[i], EXP, scale=0.125)
                        pts[kt] = pt2
                    if g == 1:
                        if finish_prev is not None:
                            finish_prev()
                        if push_after is not None:
                            push_after()
                    if g > 0:
                        for kt in (2 * g - 2, 2 * g - 1):
                            nc.tensor.matmul(pvA, v1[kt], pts[kt][:, 0:512],
                                             start=(kt == 0), stop=False)
                            nc.tensor.matmul(pvB, v1[kt], pts[kt][:, 512:1024],
                                             start=(kt == 0), stop=False)
                            del pts[kt]
                    if g < 6:
                        pull(1100)
                for kt in (14, 15):
                    nc.tensor.matmul(pvA, v1[kt], pts[kt][:, 0:512],
                                     start=False, stop=(kt == 15))
                    nc.tensor.matmul(pvB, v1[kt], pts[kt][:, 512:1024],
                                     start=False, stop=(kt == 15))
                    del pts[kt]

                def finish():
                    # custom-DVE recip reads garbage from PSUM: stage via SBUF
                    for sub, pv in ((0, pvA), (1, pvB)):
                        nc.vector.tensor_copy(lsb[sub], pv[64:65, :])
                    for sub in (0, 1):
                        nc.vector.reciprocal_approx_fast(rcs[sub], lsb[sub])
                    for sub in (0, 1):
                        nc.vector.tensor_copy(rcb[sub][0:1, :], rcs[sub])
                    bpss, bsbs = [], []
                    for sub in (0, 1):
                        bps = pp.tile([128, 512], F32, tag="acc",
                                      name=f"b_{qc}_{hp}_{sub}")
                        nc.tensor.matmul(bps, onesr_sb, rcb[sub],
                                         start=True, stop=True)
                        bpss.append(bps)
                    for sub in (0, 1):
                        bsb = bsbp.tile([64, 512], F32, tag="bsb",
                                        name=f"bsb_{qc}_{hp}_{sub}")
                        nc.vector.tensor_copy(bsb, bpss[sub][0:64])
                        bsbs.append(bsb)
                    for sub, pv in ((0, pvA), (1, pvB)):
                        nc.vector.tensor_mul(dst[sub * 64:sub * 64 + 64, qs],
                                             pv[0:64, :], bsbs[sub])
                return finish

            # ---- schedule ----
            for sc in range(1, 4):
                push_projq_quanta(0, sc)
                push_projq_quanta(1, sc)
            fin = None
            for qc in range(4):
                pa = (lambda q=qc - 1: push_wo_quanta(q)) if qc > 0 else None
                fin = attention(qc, 0, fin, pa)
                fin = attention(qc, 1, fin, None)
            fin()
            push_wo_quanta(3)
            pull(10**9)  # flush remaining Wo work

    nc.compile()
    return nc


def _host_prep(x, Wq, Wk, Wv, Wo):
    """Build per-core input maps (host-side numpy, untimed)."""
    import ml_dtypes
    bf16 = ml_dtypes.bfloat16

    x2 = np.ascontiguousarray(x.reshape(S, D), dtype=np.float32)
    xT = np.ascontiguousarray(x2.T).astype(bf16)

    inv = 1.0 / (ROPE_BASE ** (np.arange(0, HD, 2, dtype=np.float32) / HD))
    t = np.arange(S, dtype=np.float32)
    ang = np.einsum("i,j->ij", t, inv)              # [S, 32]
    emb = np.concatenate([ang, ang], axis=-1)       # [S, 64]
    cosT = np.ascontiguousarray(np.cos(emb).T.astype(np.float32))   # [64, S]
    sinT = np.ascontiguousarray(np.sin(emb).T.astype(np.float32))
    sinTs = sinT.copy()
    sinTs[0:32] *= -1.0
    cos2 = np.concatenate([cosT, cosT], axis=0).astype(bf16)
    sin2s_base = np.concatenate([sinTs, sinTs], axis=0)
    # XOR-32 row permutation: kernel reads sin at the *source* partition of
    # each rotate-half multiply so both DVE inputs share a base partition.
    perm = np.arange(128) ^ 32
    sin2s = np.ascontiguousarray(sin2s_base[perm]).astype(bf16)

    onesr = np.zeros((128, 128), dtype=np.float32)
    onesr[0, :] = 1.0
    onesr = onesr.astype(bf16)
    ident = np.eye(64, dtype=np.float32).astype(bf16)

    in_maps = []
    for c in range(N_CORES):
        osl = slice(c * 256, (c + 1) * 256)
        ksl = slice(c * 64, (c + 1) * 64)
        # packed [D, 384] = [Wq_c^T (256) | Wk_c^T (64) | Wv_c^T (64)]
        wqkv = np.ascontiguousarray(np.concatenate(
            [Wq[osl, :], Wk[ksl, :], Wv[ksl, :]], axis=0).T).astype(bf16)
        wot = np.ascontiguousarray(Wo[:, osl].T).astype(bf16)          # [256, D]
        in_maps.append({
            "xT": xT, "wqkv": wqkv, "wot": wot,
            "cos2": cos2, "sin2s": sin2s,
            "onesr": onesr, "ident": ident,
        })
    return in_maps


def kernel(x, Wq, Wk, Wv, Wo, _trace=False):
    from concourse.bass_utils import run_bass_kernel_spmd

    x = np.asarray(x, dtype=np.float32)
    Wq = np.asarray(Wq, dtype=np.float32)
    Wk = np.asarray(Wk, dtype=np.float32)
    Wv = np.asarray(Wv, dtype=np.float32)
    Wo = np.asarray(Wo, dtype=np.float32)

    if "nc" not in _cached:
        _cached["nc"] = _build_program()
    nc = _cached["nc"]

    in_maps = _host_prep(x, Wq, Wk, Wv, Wo)
    res = run_bass_kernel_spmd(nc, in_maps, core_ids=list(range(N_CORES)),
                               trace=_trace)
    out = np.zeros((S, D), dtype=np.float64)
    for r in res.results:
        out += np.asarray(r["y"]).astype(np.float64)
    _cached["last_results"] = res
    return out.astype(np.float32).reshape(1, S, D)


# revision 17
# speedup vs baseline: 1.4220x; 1.0422x over previous
"""GQA (B=1, S=2048, D=2048, 32 Q heads / 8 KV heads, head_dim=64, RoPE,
non-causal softmax) on 8 Trainium2 NeuronCores.

Sharding: tensor-parallel over heads. Core c owns Q heads 4c..4c+3 and KV head c.
Each core computes y_c = softmax(q_c k_c^T / 8) v_c @ Wo[:, c*256:(c+1)*256].T
(a full [S, D] partial, bf16); the host sums the 8 partials.

v4:
  - bf16 SBUF operands everywhere; PSUM f32.
  - input DMA split across sync/vector/scalar HWDGE rings; x as 16 contiguous
    [128,2048] row-tiles.
  - fused lead: all 4 KV chunks + q0/q1 chunk 0 accumulate in one k-loop while
    x streams in (KV accs live in the score-pool banks, q accs in the pv-pool
    banks -- both free during the lead), so compute tracks DMA arrival.
  - attention kt-pair groups: 4 score MMs (64x128 mode), 2 exps N=1024,
    4 PV MMs (128x128); softmax bcast via [128,128] ones-row matmul.
  - deferred norm: each attention's normalization is emitted two groups into
    the NEXT attention, so the boundary never starves ACT (the ~147us exp
    stream is the floor).
  - filler queue: remaining q projections + Wo in ~0.5-0.9us quanta pulled
    between kt groups; y stored as contiguous row-tiles on alternating rings.
"""

import numpy as np

S = 2048
D = 2048
HD = 64
N_CORES = 8
ROPE_BASE = 10000.0

_cached = {}


def _build_program():
    import concourse.bass as bass
    import concourse.mybir as mybir
    import concourse.tile as tile
    from concourse import bacc

    BF16, F32 = mybir.dt.bfloat16, mybir.dt.float32
    F32R = mybir.dt.float32r
    EXP = mybir.ActivationFunctionType.Exp

    nc = bacc.Bacc("TRN2", target_bir_lowering=False, debug=False)

    xT = nc.dram_tensor("xT", [D, S], BF16, kind="ExternalInput").ap()
    wqkv = nc.dram_tensor("wqkv", [D, 384], BF16, kind="ExternalInput").ap()
    wot = nc.dram_tensor("wot", [256, D], BF16, kind="ExternalInput").ap()
    cos2 = nc.dram_tensor("cos2", [128, S], BF16, kind="ExternalInput").ap()
    sin2s = nc.dram_tensor("sin2s", [128, S], BF16, kind="ExternalInput").ap()
    onesr = nc.dram_tensor("onesr", [128, 128], BF16,
                           kind="ExternalInput").ap()
    ident = nc.dram_tensor("ident", [64, 64], BF16, kind="ExternalInput").ap()
    y = nc.dram_tensor("y", [S, D], BF16, kind="ExternalOutput").ap()

    with tile.TileContext(nc) as tc:
        with tc.tile_pool(name="singles", bufs=1) as singles, \
             tc.tile_pool(name="rope", bufs=3) as rope, \
             tc.tile_pool(name="vtcp", bufs=2) as vtcp, \
             tc.tile_pool(name="ptp", bufs=6) as ptp, \
             tc.tile_pool(name="bsbp", bufs=2) as bsbp, \
             tc.tile_pool(name="ysbp", bufs=2) as ysbp, \
             tc.tile_pool(name="pp", bufs=2, space="PSUM") as pp, \
             tc.tile_pool(name="pss", bufs=2, space="PSUM") as pss, \
             tc.tile_pool(name="pspv", bufs=2, space="PSUM") as pspv, \
             nc.allow_low_precision(reason="bf16 compute is intended"):

            # ---- static loads ----
            # gpsimd: tables then wo (SWDGE, parallel); sync: wqkv/x pairs
            # interleaved (768B/4KB rows, line rate). scalar ring stays CLEAN
            # so the exp stream is never queued behind input DMA completions.
            cos_sb = singles.tile([128, S], BF16, tag="cos")
            nc.gpsimd.dma_start(out=cos_sb, in_=cos2)
            sin_sb = singles.tile([128, S], BF16, tag="sin")
            nc.gpsimd.dma_start(out=sin_sb, in_=sin2s)
            ident_sb = singles.tile([64, 64], BF16, tag="ident")
            nc.gpsimd.dma_start(out=ident_sb, in_=ident)
            onesr_sb = singles.tile([128, 128], BF16, tag="onesr")
            nc.gpsimd.dma_start(out=onesr_sb, in_=onesr)
            wqkv_t = []
            xs = []
            for k in range(16):
                tw = singles.tile([128, 384], BF16, tag=f"wqkv{k}",
                                  name=f"wqkv{k}")
                nc.sync.dma_start(out=tw, in_=wqkv[k * 128:(k + 1) * 128, :])
                wqkv_t.append(tw)
                tx = singles.tile([128, S], BF16, tag=f"x{k}", name=f"x{k}")
                nc.sync.dma_start(out=tx, in_=xT[k * 128:(k + 1) * 128, :])
                xs.append(tx)
            wq_t = [t[:, 0:256] for t in wqkv_t]
            wkv_t = [t[:, 256:384] for t in wqkv_t]
            wo_t = []
            for i in range(2):
                t = singles.tile([128, 2048], BF16, tag=f"wo{i}", name=f"wo{i}")
                nc.gpsimd.dma_start(out=t, in_=wot[i * 128:(i + 1) * 128, :])
                wo_t.append(t)

            qTr0 = singles.tile([128, S], BF16, tag="qTr0")
            qTr1 = singles.tile([128, S], BF16, tag="qTr1")
            kTr = singles.tile([128, S], BF16, tag="kTr")  # rows 64:128 dup 0:64
            otp0 = singles.tile([128, S], BF16, tag="otp0")
            otp1 = singles.tile([128, S], BF16, tag="otp1")
            v1 = [singles.tile([128, 65], BF16, tag=f"v1_{kt}", name=f"v1_{kt}")
                  for kt in range(16)]
            for kt in range(16):
                nc.vector.memset(v1[kt][:, 64:65], 1.0)
            rcb = []
            rcs = []
            for i in range(2):
                t = singles.tile([128, 512], BF16, tag=f"rcb{i}", name=f"rcb{i}")
                nc.vector.memset(t, 0.0)
                rcb.append(t)
                t2_ = singles.tile([1, 512], F32, tag=f"rcs{i}", name=f"rcs{i}")
                rcs.append(t2_)
            lsb = [singles.tile([1, 512], F32, tag=f"lsb{i}", name=f"lsb{i}")
                   for i in range(2)]

            def ecopy(eng, out, in_):
                if eng is nc.scalar:
                    nc.scalar.copy(out, in_)
                else:
                    nc.vector.tensor_copy(out, in_)

            def rope_q(acc, dst, scs, tagix):
                t0 = rope.tile([128, 512], BF16, tag="t0", name=f"t0_{tagix}")
                nc.vector.tensor_copy(t0, acc)  # psum f32 -> sbuf bf16
                t1 = rope.tile([128, 512], BF16, tag="t1", name=f"t1_{tagix}")
                t2 = rope.tile([128, 512], BF16, tag="t2", name=f"t2_{tagix}")
                # sin_sb rows are XOR-32 permuted on host: both TT inputs share
                # base partition (verifier NCC_IBIR297); output carries shift.
                nc.vector.tensor_mul(t1, t0, cos_sb[:, scs])
                nc.vector.tensor_mul(t2[0:32], t0[32:64], sin_sb[32:64, scs])
                nc.vector.tensor_mul(t2[32:64], t0[0:32], sin_sb[0:32, scs])
                nc.vector.tensor_mul(t2[64:96], t0[96:128], sin_sb[96:128, scs])
                nc.vector.tensor_mul(t2[96:128], t0[64:96], sin_sb[64:96, scs])
                nc.vector.tensor_add(dst[:, scs], t1, t2)

            def kv_post(acc, sc, cp):
                """RoPE-K + kTr dup + v transposes for one 512-col kv chunk.
                cp: engine for the copies (nc.scalar early, nc.vector late)."""
                scs = slice(sc * 512, (sc + 1) * 512)
                t0 = rope.tile([128, 512], BF16, tag="t0", name=f"t0_kv{sc}")
                nc.vector.tensor_copy(t0[0:64], acc[0:64])
                t1 = rope.tile([128, 512], BF16, tag="t1", name=f"t1_kv{sc}")
                t2 = rope.tile([128, 512], BF16, tag="t2", name=f"t2_kv{sc}")
                nc.vector.tensor_mul(t1[0:64], t0[0:64], cos_sb[0:64, scs])
                nc.vector.tensor_mul(t2[0:32], t0[32:64], sin_sb[32:64, scs])
                nc.vector.tensor_mul(t2[32:64], t0[0:32], sin_sb[0:32, scs])
                nc.vector.tensor_add(kTr[0:64, scs], t1[0:64], t2[0:64])
                ecopy(cp, kTr[64:128, scs], kTr[0:64, scs])
                vtc = vtcp.tile([64, 512], BF16, tag="vtc", name=f"vtc{sc}")
                ecopy(cp, vtc, acc[64:128])
                for b in range(4):
                    kt = sc * 4 + b
                    tp = pspv.tile([128, 64], BF16, tag="pv", name=f"tp_{kt}")
                    nc.tensor.transpose(tp, vtc[:, b * 128:(b + 1) * 128],
                                        ident_sb)
                    ecopy(cp, v1[kt][:, 0:64], tp)

            # ---- fused lead: kv(sc0..3) + q(sc0) + q(sc1) in one k-loop;
            # q accs live in the pv-pool (sc0) and pp-pool (sc1) banks.
            skv = [pss.tile([128, 1024], F32, tag="s", name=f"skv{i}")
                   for i in range(2)]
            qacc = [pspv.tile([128, 512], F32, tag="pv", name=f"qacc{ot}")
                    for ot in range(2)]
            qacc2 = [pp.tile([128, 512], F32, tag="acc", name=f"qacc2_{ot}")
                     for ot in range(2)]
            for k in range(16):
                st, sp = (k == 0), (k == 15)
                for sc in range(4):
                    nc.tensor.matmul(
                        skv[sc // 2][:, (sc % 2) * 512:(sc % 2) * 512 + 512],
                        wkv_t[k], xs[k][:, sc * 512:(sc + 1) * 512],
                        start=st, stop=sp)
                for ot in range(2):
                    nc.tensor.matmul(qacc[ot],
                                     wq_t[k][:, ot * 128:(ot + 1) * 128],
                                     xs[k][:, 0:512], start=st, stop=sp)
                    nc.tensor.matmul(qacc2[ot],
                                     wq_t[k][:, ot * 128:(ot + 1) * 128],
                                     xs[k][:, 512:1024], start=st, stop=sp)
            # post-lead ordering tuned so each consumer meets its deadline:
            # scores g0 needs qTr0(sc0)+kTr(sc0); g2k needs kTr(sc_k) by
            # ~2.3us*2k; q1(sc0) by attn(0,1); q(sc1) by attn(1,*).
            # both sc0 ropes BEFORE kv_post: the transposes reuse the
            # qacc pv-pool buffers, so their consumers must be emitted first.
            rope_q(qacc[0], qTr0, slice(0, 512), "q0_0")
            rope_q(qacc[1], qTr1, slice(0, 512), "q1_0")
            kv_post(skv[0][:, 0:512], 0, nc.scalar)
            kv_post(skv[0][:, 512:1024], 1, nc.scalar)
            kv_post(skv[1][:, 0:512], 2, nc.vector)
            kv_post(skv[1][:, 512:1024], 3, nc.vector)
            rope_q(qacc2[0], qTr0, slice(512, 1024), "q0_1")
            rope_q(qacc2[1], qTr1, slice(512, 1024), "q1_1")

            # ---- filler queue ----
            filler = []

            def push_projq_quanta(ot, sc):
                scs = slice(sc * 512, (sc + 1) * 512)
                box = {}

                def mk(kq):
                    def run():
                        if kq == 0:
                            box["acc"] = pp.tile([128, 512], F32, tag="acc",
                                                 name=f"acc_{ot}_{sc}")
                        for k in range(kq * 2, kq * 2 + 2):
                            nc.tensor.matmul(
                                box["acc"],
                                wq_t[k][:, ot * 128:(ot + 1) * 128],
                                xs[k][:, scs],
                                start=(k == 0), stop=(k == 15))
                        if kq == 7:
                            rope_q(box["acc"], qTr0 if ot == 0 else qTr1,
                                   scs, f"{ot}_{sc}")
                    return run
                for kq in range(8):
                    filler.append((430, mk(kq)))

            def push_wo_quanta(qc):
                boxes = {}

                def mk(st, mc):
                    sabs = qc * 4 + st
                    ss = slice(sabs * 128, (sabs + 1) * 128)
                    ms = slice(mc * 512, (mc + 1) * 512)

                    def run():
                        if mc == 0:
                            boxes[st] = ysbp.tile([128, 2048], BF16, tag="y",
                                                  name=f"ysb_{sabs}")
                        yps = pp.tile([128, 512], F32, tag="acc",
                                      name=f"y_{sabs}_{mc}")
                        nc.tensor.matmul(yps, otp0[:, ss], wo_t[0][:, ms],
                                         start=True, stop=False)
                        nc.tensor.matmul(yps, otp1[:, ss], wo_t[1][:, ms],
                                         start=False, stop=True)
                        if qc == 3:
                            nc.scalar.copy(boxes[st][:, ms], yps)
                        else:
                            nc.vector.tensor_copy(boxes[st][:, ms], yps)
                        if mc == 3:
                            eng = nc.gpsimd if sabs % 2 == 0 else nc.sync
                            eng.dma_start(out=y[ss, :], in_=boxes[st])
                    return run
                for st in range(4):
                    for mc in range(4):
                        filler.append((480, mk(st, mc)))

            def pull(budget_ns):
                spent = 0
                while filler and spent < budget_ns:
                    cost, run = filler.pop(0)
                    run()
                    spent += cost

            # ---- attention; norm is returned as a closure emitted later ----
            def attention(qc, hp, finish_prev=None, push_after=None):
                qs = slice(qc * 512, (qc + 1) * 512)
                qsrc = qTr0 if hp == 0 else qTr1
                dst = otp0 if hp == 0 else otp1
                pvA = pspv.tile([65, 512], F32, tag="pv", name=f"pvA_{qc}_{hp}")
                pvB = pspv.tile([65, 512], F32, tag="pv", name=f"pvB_{qc}_{hp}")
                pts = {}
                for g in range(8):  # kt pairs
                    s2s = []
                    for kt in (2 * g, 2 * g + 1):
                        ktc = slice(kt * 128, (kt + 1) * 128)
                        s2 = pss.tile([128, 1024], F32, tag="s",
                                      name=f"s_{qc}_{hp}_{kt}")
                        nc.tensor.matmul(s2[:, 0:512], kTr[0:64, ktc],
                                         qsrc[0:64, qs], start=True, stop=True)
                        nc.tensor.matmul(s2[:, 512:1024], kTr[64:128, ktc],
                                         qsrc[64:128, qs], start=True, stop=True)
                        s2s.append(s2)
                    for i, kt in enumerate((2 * g, 2 * g + 1)):
                        pt2 = ptp.tile([128, 1024], BF16, tag="pt",
                                       name=f"pt_{qc}_{hp}_{kt}")
                        nc.scalar.activation(pt2, s2s[i], EXP, scale=0.125)
                        pts[kt] = pt2
                    if g == 1:
                        if finish_prev is not None:
                            finish_prev()
                        if push_after is not None:
                            push_after()
                    if g > 0:
                        for kt in (2 * g - 2, 2 * g - 1):
                            nc.tensor.matmul(pvA, v1[kt], pts[kt][:, 0:512],
                                             start=(kt == 0), stop=False)
                            nc.tensor.matmul(pvB, v1[kt], pts[kt][:, 512:1024],
                                             start=(kt == 0), stop=False)
                            del pts[kt]
                    if g < 6 and not (qc == 0 and hp == 0):
                        pull(800)
                for kt in (14, 15):
                    nc.tensor.matmul(pvA, v1[kt], pts[kt][:, 0:512],
                                     start=False, stop=(kt == 15))
                    nc.tensor.matmul(pvB, v1[kt], pts[kt][:, 512:1024],
                                     start=False, stop=(kt == 15))
                    del pts[kt]

                def finish(tail=False):
                    cp = nc.scalar if tail else nc.vector
                    # custom-DVE recip reads garbage from PSUM: stage via SBUF
                    for sub, pv in ((0, pvA), (1, pvB)):
                        ecopy(cp, lsb[sub], pv[64:65, :])
                    for sub in (0, 1):
                        nc.vector.reciprocal_approx_fast(rcs[sub], lsb[sub])
                    for sub in (0, 1):
                        nc.vector.tensor_copy(rcb[sub][0:1, :], rcs[sub])
                    bpss, bsbs = [], []
                    for sub in (0, 1):
                        bps = pp.tile([128, 512], F32, tag="acc",
                                      name=f"b_{qc}_{hp}_{sub}")
                        nc.tensor.matmul(bps, onesr_sb, rcb[sub],
                                         start=True, stop=True)
                        bpss.append(bps)
                    for sub in (0, 1):
                        bsb = bsbp.tile([64, 512], F32, tag="bsb",
                                        name=f"bsb_{qc}_{hp}_{sub}")
                        if tail:
                            nc.scalar.copy(bsb, bpss[sub][0:64])
                        else:
                            nc.vector.tensor_copy(bsb, bpss[sub][0:64])
                        bsbs.append(bsb)
                    for sub, pv in ((0, pvA), (1, pvB)):
                        nc.vector.tensor_mul(dst[sub * 64:sub * 64 + 64, qs],
                                             pv[0:64, :], bsbs[sub])
                return finish

            # ---- schedule ----
            for sc in range(2, 4):
                push_projq_quanta(0, sc)
                push_projq_quanta(1, sc)
            fin = None
            for qc in range(4):
                pa = (lambda q=qc - 1: push_wo_quanta(q)) if qc > 0 else None
                fin = attention(qc, 0, fin, pa)
                fin = attention(qc, 1, fin, None)
            fin(tail=True)
            push_wo_quanta(3)
            pull(10**9)  # flush remaining Wo work

    nc.compile()
    return nc


def _host_prep(x, Wq, Wk, Wv, Wo):
    """Build per-core input maps (host-side numpy, untimed)."""
    import ml_dtypes
    bf16 = ml_dtypes.bfloat16

    x2 = np.ascontiguousarray(x.reshape(S, D), dtype=np.float32)
    xT = np.ascontiguousarray(x2.T).astype(bf16)

    inv = 1.0 / (ROPE_BASE ** (np.arange(0, HD, 2, dtype=np.float32) / HD))
    t = np.arange(S, dtype=np.float32)
    ang = np.einsum("i,j->ij", t, inv)              # [S, 32]
    emb = np.concatenate([ang, ang], axis=-1)       # [S, 64]
    cosT = np.ascontiguousarray(np.cos(emb).T.astype(np.float32))   # [64, S]
    sinT = np.ascontiguousarray(np.sin(emb).T.astype(np.float32))
    sinTs = sinT.copy()
    sinTs[0:32] *= -1.0
    cos2 = np.concatenate([cosT, cosT], axis=0).astype(bf16)
    sin2s_base = np.concatenate([sinTs, sinTs], axis=0)
    # XOR-32 row permutation: kernel reads sin at the *source* partition of
    # each rotate-half multiply so both DVE inputs share a base partition.
    perm = np.arange(128) ^ 32
    sin2s = np.ascontiguousarray(sin2s_base[perm]).astype(bf16)

    onesr = np.zeros((128, 128), dtype=np.float32)
    onesr[0, :] = 1.0
    onesr = onesr.astype(bf16)
    ident = np.eye(64, dtype=np.float32).astype(bf16)

    in_maps = []
    for c in range(N_CORES):
        osl = slice(c * 256, (c + 1) * 256)
        ksl = slice(c * 64, (c + 1) * 64)
        # packed [D, 384] = [Wq_c^T (256) | Wk_c^T (64) | Wv_c^T (64)]
        wqkv = np.ascontiguousarray(np.concatenate(
            [Wq[osl, :], Wk[ksl, :], Wv[ksl, :]], axis=0).T).astype(bf16)
        wot = np.ascontiguousarray(Wo[:, osl].T).astype(bf16)          # [256, D]
        in_maps.append({
            "xT": xT, "wqkv": wqkv, "wot": wot,
            "cos2": cos2, "sin2s": sin2s,
            "onesr": onesr, "ident": ident,
        })
    return in_maps


def kernel(x, Wq, Wk, Wv, Wo, _trace=False):
    from concourse.bass_utils import run_bass_kernel_spmd

    x = np.asarray(x, dtype=np.float32)
    Wq = np.asarray(Wq, dtype=np.float32)
    Wk = np.asarray(Wk, dtype=np.float32)
    Wv = np.asarray(Wv, dtype=np.float32)
    Wo = np.asarray(Wo, dtype=np.float32)

    if "nc" not in _cached:
        _cached["nc"] = _build_program()
    nc = _cached["nc"]

    in_maps = _host_prep(x, Wq, Wk, Wv, Wo)
    res = run_bass_kernel_spmd(nc, in_maps, core_ids=list(range(N_CORES)),
                               trace=_trace)
    out = np.zeros((S, D), dtype=np.float64)
    for r in res.results:
        out += np.asarray(r["y"]).astype(np.float64)
    _cached["last_results"] = res
    return out.astype(np.float32).reshape(1, S, D)
